# revision 1
# baseline (speedup 1.0000x reference)
"""Trainium2 Bass kernel for nn_DMGCNLayer (GNN message passing layer).

Strategy (graph/data parallel over 8 NeuronCores):
  - Edges are bucketed by dst node range (6250 nodes per core) so each core
    produces a disjoint slice of the output -> no cross-core reduction.
  - Within a core, edges are ordered by (src-half, 128-node dst window) with
    uniform (max-over-cores) per-bucket budgets so that all 8 cores execute
    one identical SPMD program; shortfall is padded with self-neutralizing
    edges (their window-relative dst is 200, which matches no one-hot column).
  - h[src] is streamed per edge (host-gathered, bf16, feature-on-partition);
    the per-edge MLP1 (m1 = relu(hs@Wn1+bn1)@Wn2) runs on the tensor engine.
    An on-device transpose-mode dma_gather path (512B/node records, int16
    two-stream base split) exists behind Cfg.use_gather but is disabled: the
    custom Q7 instruction faults under the axon/PJRT execution path.
  - h[dst] and the segment-sum are realized as one-hot matmuls on the tensor
    engine (edges are sorted by dst window), accumulating in fp32 PSUM.
  - The message MLPs run in transposed-activation form in bf16 with folded
    weights: m2 = relu(eh@(0.8 W_e1) + (hs*hd)@(0.2 W_ue@W_e1) + b_e1)@W_e2.
  - The host adds the exact fp32 residual (+h) when assembling the output.
"""

import math
from contextlib import ExitStack
from dataclasses import dataclass

import numpy as np
import ml_dtypes

import concourse.bass as bass
import concourse.bacc as bacc
import concourse.mybir as mybir
import concourse.tile as tile
from concourse import bass_utils

BF16 = ml_dtypes.bfloat16
PADVAL = 200.0  # window-relative dst for pad edges; matches no iota column


@dataclass(frozen=True)
class Cfg:
    N: int = 50000
    E: int = 800000
    DN: int = 64
    H: int = 128
    NC: int = 8          # cores
    ST: int = 1024       # supertile (edges per pipeline step)
    CH: int = 8192       # edges per dma_gather call (multiple of ST)
    seg_oh_on_gpsimd: bool = False
    use_gather: bool = False

    @property
    def NR(self):  # nodes per core
        return self.N // self.NC

    @property
    def NW(self):  # 128-node windows per core
        return -(-self.NR // 128)

    @property
    def SPLIT(self):  # src half split for int16 gather indices
        return self.N // 2


CFG_FULL = Cfg()


# --------------------------------------------------------------------------
# planning (uniform across cores)
# --------------------------------------------------------------------------

@dataclass
class Plan:
    budg: np.ndarray      # [2, NW] edge budget per (src-half, window), 128-mult
    pos0: np.ndarray      # [2, NW] start position of each bucket
    ET: int               # total positions per core (multiple of ST)
    calls: list           # [(pos0, n, half)]
    wchunk: np.ndarray    # [ET//128] window id of each 128-chunk
    first_chunk: np.ndarray  # [ET//128] bool: first chunk of its (half,win) block
    last_chunk: np.ndarray   # [ET//128] bool: last chunk of its (half,win) block


def _make_plan(cfg: Cfg, src: np.ndarray, dst: np.ndarray) -> Plan:
    NR, NW = cfg.NR, cfg.NW
    core = dst // NR
    H2 = 2 if cfg.use_gather else 1
    half = (src >= cfg.SPLIT).astype(np.int64) if H2 == 2 else np.zeros(len(src), np.int64)
    win = (dst % NR) // 128

    counts = np.zeros((cfg.NC, H2, NW), np.int64)
    np.add.at(counts, (core, half, win), 1)
    budg = counts.max(axis=0)
    budg = ((budg + 127) // 128) * 128

    # pad each half's total to a multiple of ST (grow the last window's budget
    # with pure-pad chunks; pads neutralize themselves via the one-hot miss)
    H2 = budg.shape[0]
    for hh in range(H2):
        budg[hh, NW - 1] += (-budg[hh].sum()) % cfg.ST

    pos0 = np.zeros((H2, NW), np.int64)
    off = 0
    for hh in range(H2):
        for w in range(NW):
            pos0[hh, w] = off
            off += budg[hh, w]
    ET = int(off)
    assert ET % cfg.ST == 0

    calls = []
    for hh in range(H2):
        h0 = int(pos0[hh, 0])
        hlen = int(budg[hh].sum())
        o = 0
        while o < hlen:
            n = min(cfg.CH, hlen - o)
            calls.append((h0 + o, n, hh))
            o += n

    nch = ET // 128
    wchunk = np.zeros(nch, np.int64)
    first_chunk = np.zeros(nch, bool)
    last_chunk = np.zeros(nch, bool)
    for hh in range(H2):
        for w in range(NW):
            c0 = int(pos0[hh, w]) // 128
            c1 = c0 + int(budg[hh, w]) // 128
            wchunk[c0:c1] = w
            first_chunk[c0] = True
            last_chunk[c1 - 1] = True
    return Plan(budg, pos0, ET, calls, wchunk, first_chunk, last_chunk)


# --------------------------------------------------------------------------
# host-side input preparation
# --------------------------------------------------------------------------

def _prep(cfg: Cfg, inputs: dict, plan: Plan):
    h = np.asarray(inputs["h"], np.float32)
    eh = np.asarray(inputs["eh"], np.float32)
    src = np.asarray(inputs["src"]).astype(np.int64)
    dst = np.asarray(inputs["dst"]).astype(np.int64)
    W_node1 = np.asarray(inputs["W_node1"], np.float32)
    b_node1 = np.asarray(inputs["b_node1"], np.float32)
    W_node2 = np.asarray(inputs["W_node2"], np.float32)
    W_edge1 = np.asarray(inputs["W_edge1"], np.float32)
    b_edge1 = np.asarray(inputs["b_edge1"], np.float32)
    W_edge2 = np.asarray(inputs["W_edge2"], np.float32)
    W_comb = np.asarray(inputs["W_comb"], np.float32)
    W_ue = np.asarray(inputs["W_ue"], np.float32)

    NR, NW, ET = cfg.NR, cfg.NW, plan.ET

    hs_bf = h.astype(BF16)
    G1 = (np.maximum(h @ W_node1 + b_node1, 0.0) @ W_node2).astype(BF16)
    # record n = [hs(64) | G1_hi(64) | G1_lo(64) | 0(64)]; transpose-gather
    # puts hs on partitions 0:64 (slot 0), G1_hi on 64:128 (slot 0),
    # G1_lo on partitions 0:64 (slot 1) -- aligned with the m2 psum halves.
    recs = np.zeros((cfg.N, 256), BF16)
    recs[:, 0:64] = hs_bf
    recs[:, 64:128] = G1[:, 64:128]
    recs[:, 128:192] = G1[:, 0:64]

    # folded weights
    A = (0.8 * W_edge1).astype(BF16)                 # [64, H]
    W_ue1 = (0.2 * (W_ue @ W_edge1)).astype(BF16)    # [64, H]
    wzp = np.concatenate([W_ue1, A], axis=0)         # [128, H]; rows 0:64 act on p
    we2 = W_edge2.astype(BF16)
    wcomb = W_comb.astype(BF16)
    be1 = b_edge1.reshape(cfg.H, 1).astype(np.float32)
    iota_t = np.broadcast_to(np.arange(128, dtype=np.float32), (128, 128)).astype(BF16)
    iota_t = np.ascontiguousarray(iota_t)
    iota_c = np.arange(128, dtype=np.float32).reshape(128, 1)
    ones1 = np.ones((1, 128), BF16)

    core = dst // NR
    half = ((src >= cfg.SPLIT).astype(np.int64)
            if cfg.use_gather else np.zeros(len(src), np.int64))
    win = (dst % NR) // 128

    in_maps = []
    for k in range(cfg.NC):
        # fill positions: bucket edges then pads
        perm = np.full(ET, -1, np.int64)
        mask_k = core == k
        ek = np.nonzero(mask_k)[0]
        # stable order by (half, win)
        key = half[ek] * NW + win[ek]
        order = np.argsort(key, kind="stable")
        ek = ek[order]
        key = key[order]
        # position of each edge: bucket start + rank within bucket
        starts = plan.pos0[half[ek], win[ek]]
        # rank within bucket via cumcount on sorted keys
        changes = np.r_[True, key[1:] != key[:-1]]
        grp_start_idx = np.r_[0, np.nonzero(changes)[0][1:]]
        grp_id = np.cumsum(changes) - 1
        rank = np.arange(len(ek)) - grp_start_idx[grp_id]
        pos = starts + rank
        perm[pos] = ek

        valid = perm >= 0
        pe = perm[valid]

        eh_t = np.zeros((64, ET), BF16)
        eh_t[:, valid] = eh[pe].T.astype(BF16)

        if cfg.use_gather:
            # gather index values (relative to the half's base)
            idx_vals = np.zeros(ET, np.int16)
            sv = src[pe] - half[pe] * cfg.SPLIT
            assert sv.max(initial=0) < 32768
            idx_vals[valid] = sv.astype(np.int16)

        if cfg.use_gather:
            gidx16 = np.zeros((16, ET // 16), np.int16)
            for (p0, n, _hh) in plan.calls:
                blk = idx_vals[p0:p0 + n].reshape(n // 16, 16).T
                gidx16[:, p0 // 16:(p0 + n) // 16] = blk
            gidx = np.tile(gidx16, (8, 1))  # [128, ET//16]

        wrel = np.full(ET, PADVAL, np.float32)
        wrel[valid] = (dst[pe] - k * NR - win[pe] * 128).astype(np.float32)
        wrel_col = np.ascontiguousarray(
            wrel.reshape(ET // 128, 128).T).astype(np.float32)  # [128, ET//128]
        wrel_row = wrel.reshape(1, ET).astype(BF16)

        hwin = np.zeros((128, NW * 64), BF16)
        hk = h[k * NR:(k + 1) * NR].astype(BF16)           # [NR, 64]
        for w in range(NW):
            rows = hk[w * 128:(w + 1) * 128]
            hwin[:rows.shape[0], w * 64:w * 64 + 64] = rows

        im = {
            "eh_t": eh_t,
            "wrel_col": wrel_col,
            "wrel_row": wrel_row,
            "hwin": hwin,
            "wzp": wzp,
            "we2": we2,
            "wcomb": wcomb,
            "be1": be1,
            "iota_t": iota_t,
            "iota_c": iota_c,
            "ones1": ones1,
            "wn1": W_node1.astype(BF16),
            "wn2": W_node2.astype(BF16),
            "bn1": b_node1.reshape(cfg.H, 1).astype(np.float32),
        }
        if cfg.use_gather:
            im["recs"] = recs
            im["gidx"] = gidx
        else:
            sp = src[pe]
            hs_t = np.zeros((64, ET), BF16)
            hs_t[:, valid] = hs_bf[sp].T
            im["hs_t"] = hs_t
        in_maps.append(im)
    ctx = {"h": h}
    return in_maps, ctx


# --------------------------------------------------------------------------
# device program
# --------------------------------------------------------------------------

def _build(cfg: Cfg, plan: Plan) -> bacc.Bacc:
    ET, NW = plan.ET, cfg.NW
    f32 = mybir.dt.float32
    bf16 = mybir.dt.bfloat16
    i16 = mybir.dt.int16

    nc = bacc.Bacc("TRN2", target_bir_lowering=False, debug=False,
                   enable_asserts=False)

    d_eh = nc.dram_tensor("eh_t", [64, ET], bf16, kind="ExternalInput").ap()
    if cfg.use_gather:
        d_recs = nc.dram_tensor("recs", [cfg.N, 256], bf16, kind="ExternalInput").ap()
        d_gidx = nc.dram_tensor("gidx", [128, ET // 16], i16, kind="ExternalInput").ap()
    else:
        d_hst = nc.dram_tensor("hs_t", [64, ET], bf16, kind="ExternalInput").ap()
    d_wn1 = nc.dram_tensor("wn1", [64, cfg.H], bf16, kind="ExternalInput").ap()
    d_wn2 = nc.dram_tensor("wn2", [cfg.H, cfg.H], bf16, kind="ExternalInput").ap()
    d_bn1 = nc.dram_tensor("bn1", [cfg.H, 1], f32, kind="ExternalInput").ap()
    d_wrc = nc.dram_tensor("wrel_col", [128, ET // 128], f32, kind="ExternalInput").ap()
    d_wrr = nc.dram_tensor("wrel_row", [1, ET], bf16, kind="ExternalInput").ap()
    d_hwin = nc.dram_tensor("hwin", [128, NW * 64], bf16, kind="ExternalInput").ap()
    d_wzp = nc.dram_tensor("wzp", [128, cfg.H], bf16, kind="ExternalInput").ap()
    d_we2 = nc.dram_tensor("we2", [cfg.H, cfg.H], bf16, kind="ExternalInput").ap()
    d_wcomb = nc.dram_tensor("wcomb", [cfg.H, 64], bf16, kind="ExternalInput").ap()
    d_be1 = nc.dram_tensor("be1", [cfg.H, 1], f32, kind="ExternalInput").ap()
    d_iota_t = nc.dram_tensor("iota_t", [128, 128], bf16, kind="ExternalInput").ap()
    d_iota_c = nc.dram_tensor("iota_c", [128, 1], f32, kind="ExternalInput").ap()
    d_ones1 = nc.dram_tensor("ones1", [1, 128], bf16, kind="ExternalInput").ap()
    d_agg = nc.dram_tensor("agg", [128, NW * 64], f32, kind="ExternalOutput").ap()

    eq = mybir.AluOpType.is_equal
    mul = mybir.AluOpType.mult
    add = mybir.AluOpType.add
    Relu = mybir.ActivationFunctionType.Relu
    Tanh = mybir.ActivationFunctionType.Tanh

    NSTEP = ET // cfg.ST
    # map supertile -> (call index, local col offset)
    call_of_st = []
    for t in range(NSTEP):
        c0 = t * cfg.ST
        for ci, (p0, n, _hh) in enumerate(plan.calls):
            if p0 <= c0 < p0 + n:
                call_of_st.append((ci, c0 - p0))
                break
    assert len(call_of_st) == NSTEP

    with tile.TileContext(nc) as tc, ExitStack() as ctx:
        con = ctx.enter_context(tc.tile_pool(name="const", bufs=1))
        sb = ctx.enter_context(tc.tile_pool(name="sb", bufs=2))
        sohp = ctx.enter_context(tc.tile_pool(name="soh", bufs=12))
        gpool = ctx.enter_context(tc.tile_pool(name="gbuf", bufs=2))
        pers = ctx.enter_context(tc.tile_pool(name="pers", bufs=1))
        ps_a = ctx.enter_context(tc.tile_pool(name="ps_a", bufs=1, space="PSUM"))
        ps_b = ctx.enter_context(tc.tile_pool(name="ps_b", bufs=1, space="PSUM"))
        ps_hd = ctx.enter_context(tc.tile_pool(name="ps_hd", bufs=1, space="PSUM"))
        ps_bc = ctx.enter_context(tc.tile_pool(name="ps_bc", bufs=1, space="PSUM"))
        ps_mn = ctx.enter_context(tc.tile_pool(name="ps_mn", bufs=1, space="PSUM"))
        ps_ag = ctx.enter_context(tc.tile_pool(name="ps_ag", bufs=1, space="PSUM"))

        def load_const(tag, dram_ap, shape, dtype):
            t_ = con.tile(shape, dtype, tag=tag)
            nc.sync.dma_start(out=t_[:], in_=dram_ap)
            return t_

        c_wzp = load_const("wzp", d_wzp, [128, cfg.H], bf16)
        c_we2 = load_const("we2", d_we2, [cfg.H, cfg.H], bf16)
        c_wcomb = load_const("wcomb", d_wcomb, [cfg.H, 64], bf16)
        c_be1 = load_const("be1", d_be1, [cfg.H, 1], f32)
        c_iota_t = load_const("iota_t", d_iota_t, [128, 128], bf16)
        c_iota_c = load_const("iota_c", d_iota_c, [128, 1], f32)
        c_ones1 = load_const("ones1", d_ones1, [1, 128], bf16)
        c_hwin = load_const("hwin", d_hwin, [128, NW * 64], bf16)
        if cfg.use_gather:
            c_gidx = load_const("gidx", d_gidx, [128, ET // 16], i16)
        c_wrc = load_const("wrc", d_wrc, [128, ET // 128], f32)
        c_wn1 = load_const("wn1", d_wn1, [64, cfg.H], bf16)
        c_wn2 = load_const("wn2", d_wn2, [cfg.H, cfg.H], bf16)
        c_bn1 = load_const("bn1", d_bn1, [cfg.H, 1], f32)

        agg_sb = pers.tile([128, NW * 64], f32)
        aggp = ps_ag.tile([128, 8, 64], f32)  # rotating window accumulators

        gtiles = {}

        seg_eng = nc.gpsimd if cfg.seg_oh_on_gpsimd else nc.vector

        for t in range(NSTEP):
            if cfg.use_gather:
                ci, loc = call_of_st[t]
                if loc == 0:
                    p0, n, hh = plan.calls[ci]
                    gt = gpool.tile([128, 2, n], bf16, tag="gbuf")
                    src_ap = d_recs if hh == 0 else d_recs[cfg.SPLIT:, :]
                    nc.gpsimd.dma_gather(
                        out_ap=gt[:, :, :],
                        in_ap=src_ap,
                        idxs_ap=c_gidx[:, p0 // 16:(p0 + n) // 16],
                        num_idxs=n,
                        num_idxs_reg=n,
                        elem_size=256,
                        transpose=True,
                    )
                    gtiles[ci] = gt
                gt = gtiles[ci]
                hs_src = gt[0:64, 0, :]
                gofs = loc
            else:
                hsb = gpool.tile([64, cfg.ST], bf16, tag="hst")
                nc.sync.dma_start(out=hsb[:],
                                  in_=d_hst[:, t * cfg.ST:(t + 1) * cfg.ST])
                hs_src = hsb[:, :]
                gofs = 0

            # per-edge MLP1: m1 = relu(hs@Wn1 + bn1)@Wn2, in transposed form
            z1 = ps_a.tile([128, cfg.ST], f32, tag="za")
            for hhalf in range(cfg.ST // 512):
                cl0 = hhalf * 512
                nc.tensor.matmul(z1[:, cl0:cl0 + 512], c_wn1[:],
                                 hs_src[:, gofs + cl0:gofs + cl0 + 512],
                                 start=True, stop=True)
            r1 = sb.tile([128, cfg.ST], bf16, tag="r1")
            nc.vector.tensor_scalar(r1[:], z1[:], c_bn1[:, 0:1], 0.0,
                                    mybir.AluOpType.add, mybir.AluOpType.max)
            m1p = ps_b.tile([128, cfg.ST], f32, tag="zb")
            for hhalf in range(cfg.ST // 512):
                cl0 = hhalf * 512
                nc.tensor.matmul(m1p[:, cl0:cl0 + 512], c_wn2[:],
                                 r1[:, cl0:cl0 + 512], start=True, stop=True)
            m1sb = sb.tile([128, cfg.ST], bf16, tag="m1sb")
            nc.vector.tensor_copy(out=m1sb[:], in_=m1p[:])

            stack = sb.tile([128, cfg.ST], bf16, tag="stack")
            nc.sync.dma_start(out=stack[64:128, :],
                              in_=d_eh[:, t * cfg.ST:(t + 1) * cfg.ST])
            wrr = sb.tile([1, cfg.ST], bf16, tag="wrr")
            nc.sync.dma_start(out=wrr[:], in_=d_wrr[:, t * cfg.ST:(t + 1) * cfg.ST])

            # per-128-chunk segment one-hot [edge, node-in-window]
            seg_ohs = []
            for j in range(cfg.ST // 128):
                c = t * (cfg.ST // 128) + j
                so = sohp.tile([128, 128], bf16, tag="soh")
                seg_eng.tensor_scalar(so[:], c_iota_t[:], c_wrc[:, c:c + 1],
                                      None, eq)
                seg_ohs.append(so)

            # hd via one-hot matmul, in 512-col halves
            for hhalf in range(cfg.ST // 512):
                cl0 = hhalf * 512
                bc = ps_bc.tile([128, 512], f32, tag="bc")
                nc.tensor.matmul(bc[:], c_ones1[:],
                                 wrr[:, cl0:cl0 + 512], start=True, stop=True)
                ohT = sb.tile([128, 512], bf16, tag="ohT")
                nc.vector.tensor_scalar(ohT[:], bc[:], c_iota_c[:], None, eq)
                hd = ps_hd.tile([64, 512], f32, tag="hd")
                # window-parts inside this half (chunks are window-pure)
                j0 = cl0 // 128
                parts = []
                for j in range(j0, j0 + 4):
                    c = t * (cfg.ST // 128) + j
                    w = int(plan.wchunk[c])
                    if parts and parts[-1][2] == w:
                        parts[-1][1] += 128
                    else:
                        parts.append([j * 128 - cl0, 128, w])
                for (o, wd, w) in parts:
                    nc.tensor.matmul(hd[:, o:o + wd],
                                     c_hwin[:, w * 64:(w + 1) * 64],
                                     ohT[:, o:o + wd], start=True, stop=True)
                # p = hs * hd  -> stack partitions 0:64
                nc.vector.tensor_tensor(
                    out=stack[0:64, cl0:cl0 + 512],
                    in0=hs_src[:, gofs + cl0:gofs + cl0 + 512],
                    in1=hd[:, :], op=mul)

            z = ps_a.tile([128, cfg.ST], f32, tag="za")
            for hhalf in range(cfg.ST // 512):
                cl0 = hhalf * 512
                nc.tensor.matmul(z[:, cl0:cl0 + 512], c_wzp[:],
                                 stack[:, cl0:cl0 + 512], start=True, stop=True)
            rz = sb.tile([128, cfg.ST], bf16, tag="rz")
            nc.scalar.activation(rz[:], z[:], Relu, bias=c_be1[:, 0:1])

            m2 = ps_b.tile([128, cfg.ST], f32, tag="zb")
            for hhalf in range(cfg.ST // 512):
                cl0 = hhalf * 512
                nc.tensor.matmul(m2[:, cl0:cl0 + 512], c_we2[:],
                                 rz[:, cl0:cl0 + 512], start=True, stop=True)

            m2c = sb.tile([128, cfg.ST], bf16, tag="m2c")
            nc.scalar.activation(m2c[:], m2[:],
                                 mybir.ActivationFunctionType.Copy)
            q = sb.tile([128, cfg.ST], bf16, tag="q")
            nc.gpsimd.tensor_tensor(out=q[:, :], in0=m1sb[:, :],
                                    in1=m2c[:, :], op=mul)

            mnt = ps_mn.tile([128, cfg.ST // 128, 64], f32, tag="mnt")
            for j in range(cfg.ST // 128):
                nc.tensor.matmul(mnt[:, j, :], q[:, j * 128:(j + 1) * 128],
                                 c_wcomb[:], start=True, stop=True)
            msb = sb.tile([128, cfg.ST // 128, 64], bf16, tag="msb")
            nc.scalar.activation(msb[:], mnt[:], Tanh)

            for j in range(cfg.ST // 128):
                c = t * (cfg.ST // 128) + j
                w = int(plan.wchunk[c])
                first = bool(plan.first_chunk[c])
                last = bool(plan.last_chunk[c])
                slot = w % 8
                nc.tensor.matmul(aggp[:, slot, :], seg_ohs[j][:],
                                 msb[:, j, :], start=first, stop=last)
                if last:
                    # second pass over this window (src-half B) accumulates
                    c0 = int(plan.pos0[0, w]) // 128
                    is_first_pass = c == c0 + int(plan.budg[0, w]) // 128 - 1
                    if is_first_pass:
                        nc.vector.tensor_copy(out=agg_sb[:, w * 64:(w + 1) * 64],
                                              in_=aggp[:, slot, :])
                    else:
                        nc.vector.tensor_tensor(
                            out=agg_sb[:, w * 64:(w + 1) * 64],
                            in0=agg_sb[:, w * 64:(w + 1) * 64],
                            in1=aggp[:, slot, :], op=add)

        nc.sync.dma_start(out=d_agg, in_=agg_sb[:])

    nc.compile()
    return nc


# --------------------------------------------------------------------------
# entry points
# --------------------------------------------------------------------------

def _assemble(cfg: Cfg, results, ctx):
    h = ctx["h"]
    out = np.empty((cfg.N, cfg.DN), np.float32)
    for k in range(cfg.NC):
        agg = np.asarray(results[k]["agg"], np.float32)
        agg = agg.reshape(128, cfg.NW, 64).transpose(1, 0, 2).reshape(cfg.NW * 128, 64)
        out[k * cfg.NR:(k + 1) * cfg.NR] = agg[:cfg.NR] + h[k * cfg.NR:(k + 1) * cfg.NR]
    return out


def run_pipeline(cfg: Cfg, inputs: dict, backend: str = "hw", want_trace: bool = False):
    src = np.asarray(inputs["src"]).astype(np.int64)
    dst = np.asarray(inputs["dst"]).astype(np.int64)
    plan = _make_plan(cfg, src, dst)
    in_maps, ctx = _prep(cfg, inputs, plan)
    nc = _build(cfg, plan)
    if backend == "sim":
        from concourse.bass_interp import CoreSim
        results = []
        for k in range(cfg.NC):
            sim = CoreSim(nc, trace=False)
            for name, arr in in_maps[k].items():
                sim.tensor(name)[:] = arr
            sim.simulate()
            results.append({"agg": np.array(sim.tensor("agg"))})
        return _assemble(cfg, results, ctx), None
    res = bass_utils.run_bass_kernel_spmd(
        nc, in_maps, core_ids=list(range(cfg.NC)), trace=want_trace)
    return _assemble(cfg, res.results, ctx), res


def kernel(**inputs) -> np.ndarray:
    out, _ = run_pipeline(CFG_FULL, inputs, backend="hw")
    return out


if __name__ == "__main__":
    # smoke test at small scale on the simulator
    cfg = Cfg(N=2048, E=8192, NC=2, ST=1024, CH=2048)
    rng = np.random.default_rng(0)
    inputs = {
        "h": rng.standard_normal((cfg.N, 64), np.float32),
        "eh": rng.standard_normal((cfg.E, 64), np.float32),
        "W_node1": rng.standard_normal((64, 128), np.float32) * 0.05,
        "b_node1": rng.standard_normal((128,), np.float32) * 0.05,
        "W_node2": rng.standard_normal((128, 128), np.float32) * 0.05,
        "W_edge1": rng.standard_normal((64, 128), np.float32) * 0.05,
        "b_edge1": rng.standard_normal((128,), np.float32) * 0.05,
        "W_edge2": rng.standard_normal((128, 128), np.float32) * 0.05,
        "W_comb": rng.standard_normal((128, 64), np.float32) * 0.05,
        "W_ue": rng.standard_normal((64, 64), np.float32) * 0.05,
        "src": rng.integers(0, cfg.N, cfg.E).astype(np.int32),
        "dst": rng.integers(0, cfg.N, cfg.E).astype(np.int32),
    }
    h, eh = inputs["h"], inputs["eh"]
    hs, hd = h[inputs["src"]], h[inputs["dst"]]
    eh_new = 0.8 * eh + 0.2 * ((hs * hd) @ inputs["W_ue"])
    m1 = np.maximum(hs @ inputs["W_node1"] + inputs["b_node1"], 0) @ inputs["W_node2"]
    m2 = np.maximum(eh_new @ inputs["W_edge1"] + inputs["b_edge1"], 0) @ inputs["W_edge2"]
    m = np.tanh((m1 * m2) @ inputs["W_comb"])
    agg = np.zeros((cfg.N, 64), np.float32)
    np.add.at(agg, inputs["dst"], m)
    expected = agg + h

    out, _ = run_pipeline(cfg, inputs, backend="sim")
    err = np.abs(out - expected)
    rel = np.abs(err).max() / np.abs(expected).max()
    print("max abs err:", err.max(), " rel(absmax):", rel)
    print("mean abs err:", err.mean())
    assert rel < 2e-2, "accuracy failure"
    print("SIM OK")



# revision 3
# speedup vs baseline: 1.9050x; 1.9050x over previous
"""Trainium2 Bass kernel for nn_DMGCNLayer (GNN message passing layer).

Strategy (graph/data parallel over 8 NeuronCores):
  - Edges are bucketed by dst node range (6250 nodes per core) so each core
    produces a disjoint slice of the output -> no cross-core reduction.
  - Within a core, edges are ordered by 128-node dst window with uniform
    (max-over-cores) per-bucket budgets so all 8 cores execute one identical
    SPMD program; shortfall is padded with self-neutralizing edges (their
    window-relative dst is 200, which matches no one-hot column).
  - The wall-clock of a run is dominated by the ~65 MB/s axon tunnel, so the
    two big per-edge streams (eh and host-gathered h[src]) are shipped as
    int8 with the dequant scales folded into the bf16 weights on the host;
    the device only widens int8 -> bf16. Metadata rides as uint8 and the
    output aggregate returns as bf16 (residual +h is added on host in fp32).
  - h[dst] and the segment-sum are realized as one-hot matmuls on the tensor
    engine (edges are sorted by dst window), accumulating in fp32 PSUM.
  - The message MLPs run in transposed-activation form in bf16 with folded
    weights: m2 = relu(eh@(0.8 W_e1) + (hs*hd)@(0.2 W_ue@W_e1) + b_e1)@W_e2.
  - The PJRT executable (jit of the shard_map'd bass custom call) is cached
    at module level so repeat runs pay only input upload + execute + fetch.
"""

import hashlib
from contextlib import ExitStack
from dataclasses import dataclass

import numpy as np
import ml_dtypes

import concourse.bass as bass
import concourse.bacc as bacc
import concourse.mybir as mybir
import concourse.tile as tile

BF16 = ml_dtypes.bfloat16
PADVAL = 200.0  # window-relative dst for pad edges; matches no iota column


@dataclass(frozen=True)
class Cfg:
    N: int = 50000
    E: int = 800000
    DN: int = 64
    H: int = 128
    NC: int = 8          # cores
    ST: int = 1024       # supertile (edges per pipeline step)

    @property
    def NR(self):  # nodes per core
        return self.N // self.NC

    @property
    def NW(self):  # 128-node windows per core
        return -(-self.NR // 128)


CFG_FULL = Cfg()


# --------------------------------------------------------------------------
# planning (uniform across cores)
# --------------------------------------------------------------------------

@dataclass
class Plan:
    budg: np.ndarray      # [NW] edge budget per window, 128-mult
    pos0: np.ndarray      # [NW] start position of each bucket
    ET: int               # total positions per core (multiple of ST)
    wchunk: np.ndarray    # [ET//128] window id of each 128-chunk
    first_chunk: np.ndarray  # [ET//128] bool: first chunk of its window block
    last_chunk: np.ndarray   # [ET//128] bool: last chunk of its window block


def _make_plan(cfg: Cfg, src: np.ndarray, dst: np.ndarray) -> Plan:
    NR, NW = cfg.NR, cfg.NW
    core = dst // NR
    win = (dst % NR) // 128

    counts = np.zeros((cfg.NC, NW), np.int64)
    np.add.at(counts, (core, win), 1)
    budg = counts.max(axis=0)
    budg = ((budg + 127) // 128) * 128
    budg[NW - 1] += (-budg.sum()) % cfg.ST

    pos0 = np.zeros(NW, np.int64)
    off = 0
    for w in range(NW):
        pos0[w] = off
        off += budg[w]
    ET = int(off)
    assert ET % cfg.ST == 0

    nch = ET // 128
    wchunk = np.zeros(nch, np.int64)
    first_chunk = np.zeros(nch, bool)
    last_chunk = np.zeros(nch, bool)
    for w in range(NW):
        c0 = int(pos0[w]) // 128
        c1 = c0 + int(budg[w]) // 128
        wchunk[c0:c1] = w
        first_chunk[c0] = True
        last_chunk[c1 - 1] = True
    return Plan(budg, pos0, ET, wchunk, first_chunk, last_chunk)


# --------------------------------------------------------------------------
# host-side input preparation
# --------------------------------------------------------------------------

def _prep(cfg: Cfg, inputs: dict, plan: Plan):
    """Build the global (concat-over-cores along axis 0) input arrays."""
    h = np.asarray(inputs["h"], np.float32)
    eh = np.asarray(inputs["eh"], np.float32)
    src = np.asarray(inputs["src"]).astype(np.int64)
    dst = np.asarray(inputs["dst"]).astype(np.int64)
    W_node1 = np.asarray(inputs["W_node1"], np.float32)
    b_node1 = np.asarray(inputs["b_node1"], np.float32)
    W_node2 = np.asarray(inputs["W_node2"], np.float32)
    W_edge1 = np.asarray(inputs["W_edge1"], np.float32)
    b_edge1 = np.asarray(inputs["b_edge1"], np.float32)
    W_edge2 = np.asarray(inputs["W_edge2"], np.float32)
    W_comb = np.asarray(inputs["W_comb"], np.float32)
    W_ue = np.asarray(inputs["W_ue"], np.float32)

    NR, NW, ET, NC = cfg.NR, cfg.NW, plan.ET, cfg.NC

    # int8 quantization of the two big per-edge streams
    s_h = float(np.abs(h).max()) / 127.0
    s_e = float(np.abs(eh).max()) / 127.0
    q_h = np.clip(np.round(h / s_h), -127, 127).astype(np.int8)      # [N, 64]
    q_eh = np.clip(np.round(eh / s_e), -127, 127).astype(np.int8)    # [E, 64]

    # folded weights (dequant scales folded in)
    A = (s_e * 0.8 * W_edge1).astype(BF16)                # acts on int8 eh
    W_ue1 = (s_h * 0.2 * (W_ue @ W_edge1)).astype(BF16)   # acts on q_hs * hd
    wzp = np.concatenate([W_ue1, A], axis=0)              # [128, H]
    wn1 = (s_h * W_node1).astype(BF16)                    # acts on int8 hs

    iota_t = np.ascontiguousarray(
        np.broadcast_to(np.arange(128, dtype=np.float32), (128, 128))).astype(BF16)
    iota_c = np.arange(128, dtype=np.float32).reshape(128, 1)
    ones1 = np.ones((1, 128), BF16)

    core = dst // NR
    win = (dst % NR) // 128

    g_eh = np.zeros((NC * 64, ET), np.int8)
    g_hs = np.zeros((NC * 64, ET), np.int8)
    g_wrc = np.full((NC * 128, ET // 128), int(PADVAL), np.uint8)
    g_wrr = np.full((NC * 1, ET), int(PADVAL), np.uint8)
    g_hwin = np.zeros((NC * 128, NW * 64), BF16)

    for k in range(NC):
        perm = np.full(ET, -1, np.int64)
        ek = np.nonzero(core == k)[0]
        key = win[ek]
        order = np.argsort(key, kind="stable")
        ek = ek[order]
        key = key[order]
        starts = plan.pos0[key]
        changes = np.r_[True, key[1:] != key[:-1]]
        grp_start_idx = np.r_[0, np.nonzero(changes)[0][1:]]
        grp_id = np.cumsum(changes) - 1
        rank = np.arange(len(ek)) - grp_start_idx[grp_id]
        perm[starts + rank] = ek

        valid = perm >= 0
        pe = perm[valid]

        g_eh[k * 64:(k + 1) * 64, valid] = q_eh[pe].T
        g_hs[k * 64:(k + 1) * 64, valid] = q_h[src[pe]].T

        wrel = np.full(ET, int(PADVAL), np.uint8)
        wrel[valid] = (dst[pe] - k * NR - win[pe] * 128).astype(np.uint8)
        g_wrc[k * 128:(k + 1) * 128] = np.ascontiguousarray(
            wrel.reshape(ET // 128, 128).T)
        g_wrr[k] = wrel

        hk = h[k * NR:(k + 1) * NR].astype(BF16)
        for w in range(NW):
            rows = hk[w * 128:(w + 1) * 128]
            g_hwin[k * 128:k * 128 + rows.shape[0], w * 64:w * 64 + 64] = rows

    def rep(x):  # replicate a shared tensor across cores along axis 0
        return np.ascontiguousarray(np.tile(x, (NC,) + (1,) * (x.ndim - 1)))

    gmap = {
        "eh_t": g_eh,
        "hs_t": g_hs,
        "wrel_col": g_wrc,
        "wrel_row": g_wrr,
        "hwin": g_hwin,
        "wzp": rep(wzp),
        "we2": rep(W_edge2.astype(BF16)),
        "wcomb": rep(W_comb.astype(BF16)),
        "be1": rep(b_edge1.reshape(cfg.H, 1).astype(np.float32)),
        "iota_t": rep(iota_t),
        "iota_c": rep(iota_c),
        "ones1": rep(ones1),
        "wn1": rep(wn1),
        "wn2": rep(W_node2.astype(BF16)),
        "bn1": rep(b_node1.reshape(cfg.H, 1).astype(np.float32)),
    }
    ctx = {"h": h}
    return gmap, ctx


# --------------------------------------------------------------------------
# device program
# --------------------------------------------------------------------------

def _build(cfg: Cfg, plan: Plan) -> bacc.Bacc:
    ET, NW = plan.ET, cfg.NW
    f32 = mybir.dt.float32
    bf16 = mybir.dt.bfloat16
    i8 = mybir.dt.int8
    u8 = mybir.dt.uint8

    nc = bacc.Bacc("TRN2", target_bir_lowering=False, debug=False,
                   enable_asserts=False)

    d_eh = nc.dram_tensor("eh_t", [64, ET], i8, kind="ExternalInput").ap()
    d_hst = nc.dram_tensor("hs_t", [64, ET], i8, kind="ExternalInput").ap()
    d_wn1 = nc.dram_tensor("wn1", [64, cfg.H], bf16, kind="ExternalInput").ap()
    d_wn2 = nc.dram_tensor("wn2", [cfg.H, cfg.H], bf16, kind="ExternalInput").ap()
    d_bn1 = nc.dram_tensor("bn1", [cfg.H, 1], f32, kind="ExternalInput").ap()
    d_wrc = nc.dram_tensor("wrel_col", [128, ET // 128], u8, kind="ExternalInput").ap()
    d_wrr = nc.dram_tensor("wrel_row", [1, ET], u8, kind="ExternalInput").ap()
    d_hwin = nc.dram_tensor("hwin", [128, NW * 64], bf16, kind="ExternalInput").ap()
    d_wzp = nc.dram_tensor("wzp", [128, cfg.H], bf16, kind="ExternalInput").ap()
    d_we2 = nc.dram_tensor("we2", [cfg.H, cfg.H], bf16, kind="ExternalInput").ap()
    d_wcomb = nc.dram_tensor("wcomb", [cfg.H, 64], bf16, kind="ExternalInput").ap()
    d_be1 = nc.dram_tensor("be1", [cfg.H, 1], f32, kind="ExternalInput").ap()
    d_iota_t = nc.dram_tensor("iota_t", [128, 128], bf16, kind="ExternalInput").ap()
    d_iota_c = nc.dram_tensor("iota_c", [128, 1], f32, kind="ExternalInput").ap()
    d_ones1 = nc.dram_tensor("ones1", [1, 128], bf16, kind="ExternalInput").ap()
    d_agg = nc.dram_tensor("agg", [128, NW * 64], bf16, kind="ExternalOutput").ap()

    eq = mybir.AluOpType.is_equal
    mul = mybir.AluOpType.mult
    add = mybir.AluOpType.add
    Relu = mybir.ActivationFunctionType.Relu
    Tanh = mybir.ActivationFunctionType.Tanh

    NSTEP = ET // cfg.ST

    with tile.TileContext(nc) as tc, ExitStack() as ctx:
        con = ctx.enter_context(tc.tile_pool(name="const", bufs=1))
        sb = ctx.enter_context(tc.tile_pool(name="sb", bufs=2))
        sohp = ctx.enter_context(tc.tile_pool(name="soh", bufs=12))
        gpool = ctx.enter_context(tc.tile_pool(name="gbuf", bufs=2))
        pers = ctx.enter_context(tc.tile_pool(name="pers", bufs=1))
        ps_a = ctx.enter_context(tc.tile_pool(name="ps_a", bufs=1, space="PSUM"))
        ps_b = ctx.enter_context(tc.tile_pool(name="ps_b", bufs=1, space="PSUM"))
        ps_hd = ctx.enter_context(tc.tile_pool(name="ps_hd", bufs=1, space="PSUM"))
        ps_bc = ctx.enter_context(tc.tile_pool(name="ps_bc", bufs=1, space="PSUM"))
        ps_mn = ctx.enter_context(tc.tile_pool(name="ps_mn", bufs=1, space="PSUM"))
        ps_ag = ctx.enter_context(tc.tile_pool(name="ps_ag", bufs=1, space="PSUM"))

        def load_const(tag, dram_ap, shape, dtype):
            t_ = con.tile(shape, dtype, tag=tag)
            nc.sync.dma_start(out=t_[:], in_=dram_ap)
            return t_

        c_wzp = load_const("wzp", d_wzp, [128, cfg.H], bf16)
        c_we2 = load_const("we2", d_we2, [cfg.H, cfg.H], bf16)
        c_wcomb = load_const("wcomb", d_wcomb, [cfg.H, 64], bf16)
        c_be1 = load_const("be1", d_be1, [cfg.H, 1], f32)
        c_iota_t = load_const("iota_t", d_iota_t, [128, 128], bf16)
        c_iota_c = load_const("iota_c", d_iota_c, [128, 1], f32)
        c_ones1 = load_const("ones1", d_ones1, [1, 128], bf16)
        c_hwin = load_const("hwin", d_hwin, [128, NW * 64], bf16)
        c_wrc8 = load_const("wrc8", d_wrc, [128, ET // 128], u8)
        c_wn1 = load_const("wn1", d_wn1, [64, cfg.H], bf16)
        c_wn2 = load_const("wn2", d_wn2, [cfg.H, cfg.H], bf16)
        c_bn1 = load_const("bn1", d_bn1, [cfg.H, 1], f32)

        c_wrc = pers.tile([128, ET // 128], f32)
        nc.vector.tensor_copy(out=c_wrc[:], in_=c_wrc8[:])

        agg_sb = pers.tile([128, NW * 64], f32)
        aggp = ps_ag.tile([128, 8, 64], f32)  # rotating window accumulators

        for t in range(NSTEP):
            hs8 = gpool.tile([64, cfg.ST], i8, tag="hs8")
            nc.sync.dma_start(out=hs8[:],
                              in_=d_hst[:, t * cfg.ST:(t + 1) * cfg.ST])
            hsb = gpool.tile([64, cfg.ST], bf16, tag="hsb")
            nc.scalar.copy(out=hsb[:], in_=hs8[:])

            # per-edge MLP1: m1 = relu(hs@Wn1 + bn1)@Wn2, in transposed form
            z1 = ps_a.tile([128, cfg.ST], f32, tag="za")
            for hhalf in range(cfg.ST // 512):
                cl0 = hhalf * 512
                nc.tensor.matmul(z1[:, cl0:cl0 + 512], c_wn1[:],
                                 hsb[:, cl0:cl0 + 512],
                                 start=True, stop=True)
            r1 = sb.tile([128, cfg.ST], bf16, tag="r1")
            nc.vector.tensor_scalar(r1[:], z1[:], c_bn1[:, 0:1], 0.0,
                                    mybir.AluOpType.add, mybir.AluOpType.max)
            m1p = ps_b.tile([128, cfg.ST], f32, tag="zb")
            for hhalf in range(cfg.ST // 512):
                cl0 = hhalf * 512
                nc.tensor.matmul(m1p[:, cl0:cl0 + 512], c_wn2[:],
                                 r1[:, cl0:cl0 + 512], start=True, stop=True)
            m1sb = sb.tile([128, cfg.ST], bf16, tag="m1sb")
            nc.vector.tensor_copy(out=m1sb[:], in_=m1p[:])

            stack = sb.tile([128, cfg.ST], bf16, tag="stack")
            eh8 = gpool.tile([64, cfg.ST], i8, tag="eh8")
            nc.sync.dma_start(out=eh8[:],
                              in_=d_eh[:, t * cfg.ST:(t + 1) * cfg.ST])
            nc.scalar.copy(out=stack[64:128, :], in_=eh8[:])
            wrr8 = sb.tile([1, cfg.ST], u8, tag="wrr8")
            nc.sync.dma_start(out=wrr8[:], in_=d_wrr[:, t * cfg.ST:(t + 1) * cfg.ST])
            wrr = sb.tile([1, cfg.ST], bf16, tag="wrr")
            nc.vector.tensor_copy(out=wrr[:], in_=wrr8[:])

            # per-128-chunk segment one-hot [edge, node-in-window]
            seg_ohs = []
            for j in range(cfg.ST // 128):
                c = t * (cfg.ST // 128) + j
                so = sohp.tile([128, 128], bf16, tag="soh")
                nc.vector.tensor_scalar(so[:], c_iota_t[:], c_wrc[:, c:c + 1],
                                        None, eq)
                seg_ohs.append(so)

            # hd via one-hot matmul, in 512-col halves
            for hhalf in range(cfg.ST // 512):
                cl0 = hhalf * 512
                bc = ps_bc.tile([128, 512], f32, tag="bc")
                nc.tensor.matmul(bc[:], c_ones1[:],
                                 wrr[:, cl0:cl0 + 512], start=True, stop=True)
                ohT = sb.tile([128, 512], bf16, tag="ohT")
                nc.vector.tensor_scalar(ohT[:], bc[:], c_iota_c[:], None, eq)
                hd = ps_hd.tile([64, 512], f32, tag="hd")
                # window-parts inside this half (chunks are window-pure)
                j0 = cl0 // 128
                parts = []
                for j in range(j0, j0 + 4):
                    c = t * (cfg.ST // 128) + j
                    w = int(plan.wchunk[c])
                    if parts and parts[-1][2] == w:
                        parts[-1][1] += 128
                    else:
                        parts.append([j * 128 - cl0, 128, w])
                for (o, wd, w) in parts:
                    nc.tensor.matmul(hd[:, o:o + wd],
                                     c_hwin[:, w * 64:(w + 1) * 64],
                                     ohT[:, o:o + wd], start=True, stop=True)
                # p = hs * hd  -> stack partitions 0:64
                nc.vector.tensor_tensor(
                    out=stack[0:64, cl0:cl0 + 512],
                    in0=hsb[:, cl0:cl0 + 512],
                    in1=hd[:, :], op=mul)

            z = ps_a.tile([128, cfg.ST], f32, tag="za")
            for hhalf in range(cfg.ST // 512):
                cl0 = hhalf * 512
                nc.tensor.matmul(z[:, cl0:cl0 + 512], c_wzp[:],
                                 stack[:, cl0:cl0 + 512], start=True, stop=True)
            rz = sb.tile([128, cfg.ST], bf16, tag="rz")
            nc.scalar.activation(rz[:], z[:], Relu, bias=c_be1[:, 0:1])

            m2 = ps_b.tile([128, cfg.ST], f32, tag="zb")
            for hhalf in range(cfg.ST // 512):
                cl0 = hhalf * 512
                nc.tensor.matmul(m2[:, cl0:cl0 + 512], c_we2[:],
                                 rz[:, cl0:cl0 + 512], start=True, stop=True)

            m2c = sb.tile([128, cfg.ST], bf16, tag="m2c")
            nc.scalar.activation(m2c[:], m2[:],
                                 mybir.ActivationFunctionType.Copy)
            q = sb.tile([128, cfg.ST], bf16, tag="q")
            nc.gpsimd.tensor_tensor(out=q[:, :], in0=m1sb[:, :],
                                    in1=m2c[:, :], op=mul)

            mnt = ps_mn.tile([128, cfg.ST // 128, 64], f32, tag="mnt")
            for j in range(cfg.ST // 128):
                nc.tensor.matmul(mnt[:, j, :], q[:, j * 128:(j + 1) * 128],
                                 c_wcomb[:], start=True, stop=True)
            msb = sb.tile([128, cfg.ST // 128, 64], bf16, tag="msb")
            nc.scalar.activation(msb[:], mnt[:], Tanh)

            for j in range(cfg.ST // 128):
                c = t * (cfg.ST // 128) + j
                w = int(plan.wchunk[c])
                first = bool(plan.first_chunk[c])
                last = bool(plan.last_chunk[c])
                slot = w % 8
                nc.tensor.matmul(aggp[:, slot, :], seg_ohs[j][:],
                                 msb[:, j, :], start=first, stop=last)
                if last:
                    nc.vector.tensor_copy(out=agg_sb[:, w * 64:(w + 1) * 64],
                                          in_=aggp[:, slot, :])

        agg_bf = pers.tile([128, NW * 64], bf16)
        nc.vector.tensor_copy(out=agg_bf[:], in_=agg_sb[:])
        nc.sync.dma_start(out=d_agg, in_=agg_bf[:])

    nc.compile()
    return nc


# --------------------------------------------------------------------------
# cached PJRT runner
# --------------------------------------------------------------------------

_BUNDLE: dict = {}


class _Runner:
    def __init__(self, nc, n_cores: int):
        import jax
        from jax.sharding import Mesh, PartitionSpec
        from jax.experimental.shard_map import shard_map
        from concourse import bass2jax

        bass2jax.install_neuronx_cc_hook()
        self.nc = nc
        self.n_cores = n_cores
        partition_name = (nc.partition_id_tensor.name
                          if nc.partition_id_tensor else None)
        assert nc.dbg_addr is None

        in_names, out_names, out_avals = [], [], []
        for alloc in nc.m.functions[0].allocations:
            if not isinstance(alloc, mybir.MemoryLocationSet):
                continue
            name = alloc.memorylocations[0].name
            if alloc.kind == "ExternalInput":
                if name != partition_name:
                    in_names.append(name)
            elif alloc.kind == "ExternalOutput":
                out_names.append(name)
                shape = tuple(alloc.tensor_shape)
                dtype = mybir.dt.np(alloc.dtype)
                out_avals.append(jax.core.ShapedArray(shape, dtype))
        n_params = len(in_names)
        all_names = list(in_names) + list(out_names)
        if partition_name is not None:
            all_names.append(partition_name)

        def _body(*args):
            operands = list(args)
            if partition_name is not None:
                operands.append(bass2jax.partition_id_tensor())
            outs = bass2jax._bass_exec_p.bind(
                *operands,
                out_avals=tuple(out_avals),
                in_names=tuple(all_names),
                out_names=tuple(out_names),
                lowering_input_output_aliases=(),
                sim_require_finite=True,
                sim_require_nnan=True,
                nc=nc,
            )
            return tuple(outs)

        devices = jax.devices()[:n_cores]
        mesh = Mesh(np.asarray(devices), ("core",))
        n_outs = len(out_names)
        in_specs = (PartitionSpec("core"),) * (n_params + n_outs)
        out_specs = (PartitionSpec("core"),) * n_outs
        self.fn = jax.jit(
            shard_map(_body, mesh=mesh, in_specs=in_specs,
                      out_specs=out_specs, check_rep=False),
            donate_argnums=tuple(range(n_params, n_params + n_outs)),
            keep_unused=True,
        )
        self.in_names = in_names
        self.out_names = out_names
        self.out_avals = out_avals

    def __call__(self, gmap: dict) -> dict:
        ins = [gmap[name] for name in self.in_names]
        zeros = [np.zeros((self.n_cores * a.shape[0], *a.shape[1:]), a.dtype)
                 for a in self.out_avals]
        outs = self.fn(*ins, *zeros)
        return {name: np.asarray(a) for name, a in zip(self.out_names, outs)}


def _get_runner(cfg: Cfg, plan: Plan) -> _Runner:
    hsh = hashlib.sha1()
    hsh.update(plan.budg.tobytes())
    hsh.update(plan.pos0.tobytes())
    key = (cfg, plan.ET, hsh.hexdigest())
    r = _BUNDLE.get(key)
    if r is None:
        nc = _build(cfg, plan)
        r = _Runner(nc, cfg.NC)
        _BUNDLE[key] = r
    return r


# --------------------------------------------------------------------------
# entry points
# --------------------------------------------------------------------------

def _assemble(cfg: Cfg, agg_global: np.ndarray, ctx):
    h = ctx["h"]
    out = np.empty((cfg.N, cfg.DN), np.float32)
    for k in range(cfg.NC):
        agg = agg_global[k * 128:(k + 1) * 128].astype(np.float32)
        agg = agg.reshape(128, cfg.NW, 64).transpose(1, 0, 2).reshape(cfg.NW * 128, 64)
        out[k * cfg.NR:(k + 1) * cfg.NR] = agg[:cfg.NR] + h[k * cfg.NR:(k + 1) * cfg.NR]
    return out


def run_pipeline(cfg: Cfg, inputs: dict, backend: str = "hw"):
    src = np.asarray(inputs["src"]).astype(np.int64)
    dst = np.asarray(inputs["dst"]).astype(np.int64)
    plan = _make_plan(cfg, src, dst)
    gmap, ctx = _prep(cfg, inputs, plan)
    if backend == "sim":
        from concourse.bass_interp import CoreSim
        nc = _build(cfg, plan)
        aggs = []
        for k in range(cfg.NC):
            sim = CoreSim(nc, trace=False)
            for name, arr in gmap.items():
                d0 = arr.shape[0] // cfg.NC
                sim.tensor(name)[:] = arr[k * d0:(k + 1) * d0]
            sim.simulate()
            aggs.append(np.array(sim.tensor("agg")))
        return _assemble(cfg, np.concatenate(aggs, axis=0), ctx)
    runner = _get_runner(cfg, plan)
    res = runner(gmap)
    return _assemble(cfg, res["agg"], ctx)


def kernel(**inputs) -> np.ndarray:
    return run_pipeline(CFG_FULL, inputs, backend="hw")


if __name__ == "__main__":
    # smoke test at small scale on the simulator
    cfg = Cfg(N=2048, E=8192, NC=2, ST=1024)
    rng = np.random.default_rng(0)
    inputs = {
        "h": rng.standard_normal((cfg.N, 64)).astype(np.float32),
        "eh": rng.standard_normal((cfg.E, 64)).astype(np.float32),
        "W_node1": (rng.standard_normal((64, 128)) * 0.05).astype(np.float32),
        "b_node1": (rng.standard_normal(128) * 0.05).astype(np.float32),
        "W_node2": (rng.standard_normal((128, 128)) * 0.05).astype(np.float32),
        "W_edge1": (rng.standard_normal((64, 128)) * 0.05).astype(np.float32),
        "b_edge1": (rng.standard_normal(128) * 0.05).astype(np.float32),
        "W_edge2": (rng.standard_normal((128, 128)) * 0.05).astype(np.float32),
        "W_comb": (rng.standard_normal((128, 64)) * 0.05).astype(np.float32),
        "W_ue": (rng.standard_normal((64, 64)) * 0.05).astype(np.float32),
        "src": rng.integers(0, cfg.N, cfg.E).astype(np.int32),
        "dst": rng.integers(0, cfg.N, cfg.E).astype(np.int32),
    }
    h, eh = inputs["h"], inputs["eh"]
    hs, hd = h[inputs["src"]], h[inputs["dst"]]
    eh_new = 0.8 * eh + 0.2 * ((hs * hd) @ inputs["W_ue"])
    m1 = np.maximum(hs @ inputs["W_node1"] + inputs["b_node1"], 0) @ inputs["W_node2"]
    m2 = np.maximum(eh_new @ inputs["W_edge1"] + inputs["b_edge1"], 0) @ inputs["W_edge2"]
    m = np.tanh((m1 * m2) @ inputs["W_comb"])
    agg = np.zeros((cfg.N, 64), np.float32)
    np.add.at(agg, inputs["dst"], m)
    expected = agg + h

    out = run_pipeline(cfg, inputs, backend="sim")
    err = np.abs(out - expected)
    rel = np.abs(err).max() / np.abs(expected).max()
    print("max abs err:", err.max(), " rel(absmax):", rel)
    print("mean abs err:", err.mean())
    assert rel < 2e-2, "accuracy failure"
    print("SIM OK")


# revision 8
# speedup vs baseline: 5.1814x; 2.7199x over previous
"""Trainium2 Bass kernel for nn_DMGCNLayer (GNN message passing layer).

Strategy (graph/data parallel over 8 NeuronCores):
  - Edges are bucketed by dst node range (6250 nodes per core) so each core
    produces a disjoint slice of the output -> no cross-core reduction.
  - Within a core, edges are ordered by 128-node dst window with uniform
    (max-over-cores) per-bucket budgets so all 8 cores execute one identical
    SPMD program; shortfall is padded with self-neutralizing edges (their
    window-relative dst is 200, which matches no one-hot column).
  - The wall-clock of a run is dominated by the ~65 MB/s axon tunnel, so the
    two big per-edge streams (eh and host-gathered h[src]) are shipped as
    int8 with the dequant scales folded into the bf16 weights on the host;
    the device only widens int8 -> bf16. Metadata rides as uint8 and the
    output aggregate returns as bf16 (residual +h is added on host in fp32).
  - h[dst] and the segment-sum are realized as one-hot matmuls on the tensor
    engine (edges are sorted by dst window), accumulating in fp32 PSUM.
  - The message MLPs run in transposed-activation form in bf16 with folded
    weights: m2 = relu(eh@(0.8 W_e1) + (hs*hd)@(0.2 W_ue@W_e1) + b_e1)@W_e2.
  - The PJRT executable (jit of the shard_map'd bass custom call) is cached
    at module level so repeat runs pay only input upload + execute + fetch.
"""

import hashlib
from contextlib import ExitStack
from dataclasses import dataclass

import numpy as np
import ml_dtypes

import concourse.bass as bass
import concourse.bacc as bacc
import concourse.mybir as mybir
import concourse.tile as tile

BF16 = ml_dtypes.bfloat16
PADVAL = 200.0  # window-relative dst for pad edges; matches no iota column


@dataclass(frozen=True)
class Cfg:
    N: int = 50000
    E: int = 800000
    DN: int = 64
    H: int = 128
    NC: int = 8          # cores
    ST: int = 1024       # supertile (edges per pipeline step)

    @property
    def NR(self):  # nodes per core
        return self.N // self.NC

    @property
    def NW(self):  # 128-node windows per core
        return -(-self.NR // 128)


CFG_FULL = Cfg()


# --------------------------------------------------------------------------
# planning (uniform across cores)
# --------------------------------------------------------------------------

@dataclass
class Plan:
    budg: np.ndarray      # [NW] edge budget per window, 128-mult
    pos0: np.ndarray      # [NW] start position of each bucket
    ET: int               # total positions per core (multiple of ST)
    wchunk: np.ndarray    # [ET//128] window id of each 128-chunk
    first_chunk: np.ndarray  # [ET//128] bool: first chunk of its window block
    last_chunk: np.ndarray   # [ET//128] bool: last chunk of its window block


def _make_plan(cfg: Cfg, src: np.ndarray, dst: np.ndarray) -> Plan:
    NR, NW = cfg.NR, cfg.NW
    core = dst // NR
    win = (dst % NR) // 128

    counts = np.zeros((cfg.NC, NW), np.int64)
    np.add.at(counts, (core, win), 1)
    budg = counts.max(axis=0)
    budg = ((budg + 127) // 128) * 128
    budg[NW - 1] += (-budg.sum()) % cfg.ST

    pos0 = np.zeros(NW, np.int64)
    off = 0
    for w in range(NW):
        pos0[w] = off
        off += budg[w]
    ET = int(off)
    assert ET % cfg.ST == 0

    nch = ET // 128
    wchunk = np.zeros(nch, np.int64)
    first_chunk = np.zeros(nch, bool)
    last_chunk = np.zeros(nch, bool)
    for w in range(NW):
        c0 = int(pos0[w]) // 128
        c1 = c0 + int(budg[w]) // 128
        wchunk[c0:c1] = w
        first_chunk[c0] = True
        last_chunk[c1 - 1] = True
    return Plan(budg, pos0, ET, wchunk, first_chunk, last_chunk)


# --------------------------------------------------------------------------
# host-side input preparation
# --------------------------------------------------------------------------

def _prep(cfg: Cfg, inputs: dict, plan: Plan):
    """Build the global (concat-over-cores along axis 0) input arrays."""
    h = np.asarray(inputs["h"], np.float32)
    eh = np.asarray(inputs["eh"], np.float32)
    src = np.asarray(inputs["src"]).astype(np.int64)
    dst = np.asarray(inputs["dst"]).astype(np.int64)
    W_node1 = np.asarray(inputs["W_node1"], np.float32)
    b_node1 = np.asarray(inputs["b_node1"], np.float32)
    W_node2 = np.asarray(inputs["W_node2"], np.float32)
    W_edge1 = np.asarray(inputs["W_edge1"], np.float32)
    b_edge1 = np.asarray(inputs["b_edge1"], np.float32)
    W_edge2 = np.asarray(inputs["W_edge2"], np.float32)
    W_comb = np.asarray(inputs["W_comb"], np.float32)
    W_ue = np.asarray(inputs["W_ue"], np.float32)

    NR, NW, ET, NC = cfg.NR, cfg.NW, plan.ET, cfg.NC

    # int4 quantization: per-node scales for h (src stream), per-edge for eh.
    # Device reconstructs true values ((nib-8)*s), so only the int8 h-window
    # table used for the h[dst] one-hot gather needs a weight fold (s_h).
    s_h = float(np.abs(h).max()) / 127.0
    q8_h = np.clip(np.round(h / s_h), -127, 127).astype(np.int8)     # [N, 64]
    s_hn = (np.abs(h).max(1) / 7.5).astype(BF16).astype(np.float32)  # [N]
    s_hn = np.maximum(s_hn, 1e-6)
    q4_h = (np.clip(np.round(h / s_hn[:, None]), -8, 7) + 8).astype(np.uint8)
    s_ee = (np.abs(eh).max(1) / 7.5).astype(BF16).astype(np.float32)  # [E]
    s_ee = np.maximum(s_ee, 1e-6)
    q4_eh = (np.clip(np.round(eh / s_ee[:, None]), -8, 7) + 8).astype(np.uint8)

    A = (0.8 * W_edge1).astype(BF16)                      # acts on true eh
    W_ue1 = (s_h * 0.2 * (W_ue @ W_edge1)).astype(BF16)   # acts on hs * q8_hd
    wzp = np.concatenate([W_ue1, A], axis=0)              # [128, H]
    wn1 = W_node1.astype(BF16)                            # acts on true hs

    iota_t = np.ascontiguousarray(
        np.broadcast_to(np.arange(128, dtype=np.float32), (128, 128))).astype(BF16)
    iota_c = np.arange(128, dtype=np.float32).reshape(128, 1)
    ones1 = np.ones((1, 128), BF16)

    core = dst // NR
    win = (dst % NR) // 128

    g_eh = np.zeros((NC * 64, ET // 2), np.uint8)
    g_hs = np.zeros((NC * 64, ET // 2), np.uint8)
    g_seh = np.zeros((NC * 1, ET), BF16)
    g_shs = np.zeros((NC * 1, ET), BF16)
    g_wrc = np.full((NC * 128, ET // 128), int(PADVAL), np.uint8)
    g_wrr = np.full((NC * 1, ET), int(PADVAL), np.uint8)
    g_hwin = np.zeros((NC * 128, NW * 64), np.int8)
    NSTEP = ET // cfg.ST

    def arrange_scales(s_row):
        # per supertile: first ST/2 cols = even-edge scales, rest = odd
        sr = s_row.reshape(NSTEP, cfg.ST)
        return np.concatenate([sr[:, 0::2], sr[:, 1::2]], axis=1).reshape(ET)

    for k in range(NC):
        perm = np.full(ET, -1, np.int64)
        ek = np.nonzero(core == k)[0]
        key = win[ek]
        order = np.argsort(key, kind="stable")
        ek = ek[order]
        key = key[order]
        starts = plan.pos0[key]
        changes = np.r_[True, key[1:] != key[:-1]]
        grp_start_idx = np.r_[0, np.nonzero(changes)[0][1:]]
        grp_id = np.cumsum(changes) - 1
        rank = np.arange(len(ek)) - grp_start_idx[grp_id]
        perm[starts + rank] = ek

        valid = perm >= 0
        pe = perm[valid]

        nib_e = np.full((64, ET), 8, np.uint8)
        nib_e[:, valid] = q4_eh[pe].T
        g_eh[k * 64:(k + 1) * 64] = (nib_e[:, 0::2] << 4) | nib_e[:, 1::2]
        se_row = np.ones(ET, np.float32)
        se_row[valid] = s_ee[pe]
        g_seh[k] = arrange_scales(se_row).astype(BF16)

        nib_h = np.full((64, ET), 8, np.uint8)
        nib_h[:, valid] = q4_h[src[pe]].T
        g_hs[k * 64:(k + 1) * 64] = (nib_h[:, 0::2] << 4) | nib_h[:, 1::2]
        sh_row = np.ones(ET, np.float32)
        sh_row[valid] = s_hn[src[pe]]
        g_shs[k] = arrange_scales(sh_row).astype(BF16)

        wrel = np.full(ET, int(PADVAL), np.uint8)
        wrel[valid] = (dst[pe] - k * NR - win[pe] * 128).astype(np.uint8)
        g_wrc[k * 128:(k + 1) * 128] = np.ascontiguousarray(
            wrel.reshape(ET // 128, 128).T)
        g_wrr[k] = wrel

        hk = q8_h[k * NR:(k + 1) * NR]
        for w in range(NW):
            rows = hk[w * 128:(w + 1) * 128]
            g_hwin[k * 128:k * 128 + rows.shape[0], w * 64:w * 64 + 64] = rows

    def rep(x):  # replicate a shared tensor across cores along axis 0
        return np.ascontiguousarray(np.tile(x, (NC,) + (1,) * (x.ndim - 1)))

    gmap = {
        "eh_t": g_eh,
        "hs_t": g_hs,
        "seh": g_seh,
        "shs": g_shs,
        "wrel_col": g_wrc,
        "wrel_row": g_wrr,
        "hwin": g_hwin,
        "wzp": rep(wzp),
        "we2": rep(W_edge2.astype(BF16)),
        "wcomb": rep(W_comb.astype(BF16)),
        "be1": rep(b_edge1.reshape(cfg.H, 1).astype(np.float32)),
        "iota_t": rep(iota_t),
        "iota_c": rep(iota_c),
        "ones1": rep(ones1),
        "wn1": rep(wn1),
        "wn2": rep(W_node2.astype(BF16)),
        "bn1": rep(b_node1.reshape(cfg.H, 1).astype(np.float32)),
    }
    ctx = {"h": h}
    return gmap, ctx


# --------------------------------------------------------------------------
# device program
# --------------------------------------------------------------------------

def _build(cfg: Cfg, plan: Plan) -> bacc.Bacc:
    ET, NW = plan.ET, cfg.NW
    f32 = mybir.dt.float32
    bf16 = mybir.dt.bfloat16
    i8 = mybir.dt.int8
    u8 = mybir.dt.uint8

    nc = bacc.Bacc("TRN2", target_bir_lowering=False, debug=False,
                   enable_asserts=False)

    d_eh = nc.dram_tensor("eh_t", [64, ET // 2], u8, kind="ExternalInput").ap()
    d_hst = nc.dram_tensor("hs_t", [64, ET // 2], u8, kind="ExternalInput").ap()
    d_seh = nc.dram_tensor("seh", [1, ET], bf16, kind="ExternalInput").ap()
    d_shs = nc.dram_tensor("shs", [1, ET], bf16, kind="ExternalInput").ap()
    d_wn1 = nc.dram_tensor("wn1", [64, cfg.H], bf16, kind="ExternalInput").ap()
    d_wn2 = nc.dram_tensor("wn2", [cfg.H, cfg.H], bf16, kind="ExternalInput").ap()
    d_bn1 = nc.dram_tensor("bn1", [cfg.H, 1], f32, kind="ExternalInput").ap()
    d_wrc = nc.dram_tensor("wrel_col", [128, ET // 128], u8, kind="ExternalInput").ap()
    d_wrr = nc.dram_tensor("wrel_row", [1, ET], u8, kind="ExternalInput").ap()
    d_hwin = nc.dram_tensor("hwin", [128, NW * 64], i8, kind="ExternalInput").ap()
    d_wzp = nc.dram_tensor("wzp", [128, cfg.H], bf16, kind="ExternalInput").ap()
    d_we2 = nc.dram_tensor("we2", [cfg.H, cfg.H], bf16, kind="ExternalInput").ap()
    d_wcomb = nc.dram_tensor("wcomb", [cfg.H, 64], bf16, kind="ExternalInput").ap()
    d_be1 = nc.dram_tensor("be1", [cfg.H, 1], f32, kind="ExternalInput").ap()
    d_iota_t = nc.dram_tensor("iota_t", [128, 128], bf16, kind="ExternalInput").ap()
    d_iota_c = nc.dram_tensor("iota_c", [128, 1], f32, kind="ExternalInput").ap()
    d_ones1 = nc.dram_tensor("ones1", [1, 128], bf16, kind="ExternalInput").ap()
    d_agg = nc.dram_tensor("agg", [128, NW * 64], bf16, kind="ExternalOutput").ap()

    eq = mybir.AluOpType.is_equal
    mul = mybir.AluOpType.mult
    add = mybir.AluOpType.add
    shr = mybir.AluOpType.logical_shift_right
    band = mybir.AluOpType.bitwise_and
    Relu = mybir.ActivationFunctionType.Relu
    Tanh = mybir.ActivationFunctionType.Tanh

    NSTEP = ET // cfg.ST

    with tile.TileContext(nc) as tc, ExitStack() as ctx:
        con = ctx.enter_context(tc.tile_pool(name="const", bufs=1))
        sb = ctx.enter_context(tc.tile_pool(name="sb", bufs=2))
        sohp = ctx.enter_context(tc.tile_pool(name="soh", bufs=12))
        gpool = ctx.enter_context(tc.tile_pool(name="gbuf", bufs=2))
        pers = ctx.enter_context(tc.tile_pool(name="pers", bufs=1))
        ps_a = ctx.enter_context(tc.tile_pool(name="ps_a", bufs=1, space="PSUM"))
        ps_b = ctx.enter_context(tc.tile_pool(name="ps_b", bufs=1, space="PSUM"))
        ps_hd = ctx.enter_context(tc.tile_pool(name="ps_hd", bufs=1, space="PSUM"))
        ps_bc = ctx.enter_context(tc.tile_pool(name="ps_bc", bufs=1, space="PSUM"))
        ps_mn = ctx.enter_context(tc.tile_pool(name="ps_mn", bufs=1, space="PSUM"))
        ps_ag = ctx.enter_context(tc.tile_pool(name="ps_ag", bufs=1, space="PSUM"))

        def load_const(tag, dram_ap, shape, dtype):
            t_ = con.tile(shape, dtype, tag=tag)
            nc.sync.dma_start(out=t_[:], in_=dram_ap)
            return t_

        c_wzp = load_const("wzp", d_wzp, [128, cfg.H], bf16)
        c_we2 = load_const("we2", d_we2, [cfg.H, cfg.H], bf16)
        c_wcomb = load_const("wcomb", d_wcomb, [cfg.H, 64], bf16)
        c_be1 = load_const("be1", d_be1, [cfg.H, 1], f32)
        c_iota_t = load_const("iota_t", d_iota_t, [128, 128], bf16)
        c_iota_c = load_const("iota_c", d_iota_c, [128, 1], f32)
        c_ones1 = load_const("ones1", d_ones1, [1, 128], bf16)
        c_hwin8 = load_const("hwin8", d_hwin, [128, NW * 64], i8)
        c_wrc8 = load_const("wrc8", d_wrc, [128, ET // 128], u8)
        c_wn1 = load_const("wn1", d_wn1, [64, cfg.H], bf16)
        c_wn2 = load_const("wn2", d_wn2, [cfg.H, cfg.H], bf16)
        c_bn1 = load_const("bn1", d_bn1, [cfg.H, 1], f32)

        c_wrc = pers.tile([128, ET // 128], f32)
        nc.vector.tensor_copy(out=c_wrc[:], in_=c_wrc8[:])
        c_hwin = pers.tile([128, NW * 64], bf16)
        nc.vector.tensor_copy(out=c_hwin[:], in_=c_hwin8[:])

        agg_sb = pers.tile([128, NW * 64], f32)
        aggp = ps_ag.tile([128, 8, 64], f32)  # rotating window accumulators

        HT = cfg.ST // 2
        for t in range(NSTEP):
            hs4 = gpool.tile([64, HT], u8, tag="hs4")
            nc.sync.dma_start(out=hs4[:], in_=d_hst[:, t * HT:(t + 1) * HT])
            shs = gpool.tile([1, cfg.ST], bf16, tag="shs")
            nc.sync.dma_start(out=shs[:], in_=d_shs[:, t * cfg.ST:(t + 1) * cfg.ST])
            bc_he = gpool.tile([64, HT], bf16, tag="bche")
            nc.gpsimd.partition_broadcast(bc_he[:], shs[0:1, 0:HT])
            bc_ho = gpool.tile([64, HT], bf16, tag="bcho")
            nc.gpsimd.partition_broadcast(bc_ho[:], shs[0:1, HT:cfg.ST])
            hs_hi = gpool.tile([64, HT], u8, tag="hshi")
            nc.vector.tensor_scalar(hs_hi[:], hs4[:], 4, None, shr)
            hs_lo = gpool.tile([64, HT], u8, tag="hslo")
            nc.vector.tensor_scalar(hs_lo[:], hs4[:], 15, None, band)
            hsb = gpool.tile([64, cfg.ST], bf16, tag="hsb")
            nc.vector.scalar_tensor_tensor(out=hsb[:, 0:cfg.ST:2], in0=hs_hi[:],
                                           scalar=-8.0, in1=bc_he[:],
                                           op0=add, op1=mul)
            nc.vector.scalar_tensor_tensor(out=hsb[:, 1:cfg.ST:2], in0=hs_lo[:],
                                           scalar=-8.0, in1=bc_ho[:],
                                           op0=add, op1=mul)

            # per-edge MLP1: m1 = relu(hs@Wn1 + bn1)@Wn2, in transposed form
            z1 = ps_a.tile([128, cfg.ST], f32, tag="za")
            for hhalf in range(cfg.ST // 512):
                cl0 = hhalf * 512
                nc.tensor.matmul(z1[:, cl0:cl0 + 512], c_wn1[:],
                                 hsb[:, cl0:cl0 + 512],
                                 start=True, stop=True)
            r1 = sb.tile([128, cfg.ST], bf16, tag="r1")
            nc.vector.tensor_scalar(r1[:], z1[:], c_bn1[:, 0:1], 0.0,
                                    mybir.AluOpType.add, mybir.AluOpType.max)
            m1p = ps_b.tile([128, cfg.ST], f32, tag="zb")
            for hhalf in range(cfg.ST // 512):
                cl0 = hhalf * 512
                nc.tensor.matmul(m1p[:, cl0:cl0 + 512], c_wn2[:],
                                 r1[:, cl0:cl0 + 512], start=True, stop=True)
            m1sb = sb.tile([128, cfg.ST], bf16, tag="m1sb")
            nc.vector.tensor_copy(out=m1sb[:], in_=m1p[:])

            stack = sb.tile([128, cfg.ST], bf16, tag="stack")
            eh4 = gpool.tile([64, HT], u8, tag="eh4")
            nc.sync.dma_start(out=eh4[:], in_=d_eh[:, t * HT:(t + 1) * HT])
            seh = gpool.tile([1, cfg.ST], bf16, tag="seh")
            nc.sync.dma_start(out=seh[:], in_=d_seh[:, t * cfg.ST:(t + 1) * cfg.ST])
            bc_ee = gpool.tile([64, HT], bf16, tag="bcee")
            nc.gpsimd.partition_broadcast(bc_ee[:], seh[0:1, 0:HT])
            bc_eo = gpool.tile([64, HT], bf16, tag="bceo")
            nc.gpsimd.partition_broadcast(bc_eo[:], seh[0:1, HT:cfg.ST])
            eh_hi = gpool.tile([64, HT], u8, tag="ehhi")
            nc.vector.tensor_scalar(eh_hi[:], eh4[:], 4, None, shr)
            eh_lo = gpool.tile([64, HT], u8, tag="ehlo")
            nc.vector.tensor_scalar(eh_lo[:], eh4[:], 15, None, band)
            nc.vector.scalar_tensor_tensor(out=stack[64:128, 0:cfg.ST:2],
                                           in0=eh_hi[:], scalar=-8.0,
                                           in1=bc_ee[:], op0=add, op1=mul)
            nc.vector.scalar_tensor_tensor(out=stack[64:128, 1:cfg.ST:2],
                                           in0=eh_lo[:], scalar=-8.0,
                                           in1=bc_eo[:], op0=add, op1=mul)
            wrr8 = sb.tile([1, cfg.ST], u8, tag="wrr8")
            nc.sync.dma_start(out=wrr8[:], in_=d_wrr[:, t * cfg.ST:(t + 1) * cfg.ST])
            wrr = sb.tile([1, cfg.ST], bf16, tag="wrr")
            nc.vector.tensor_copy(out=wrr[:], in_=wrr8[:])

            # per-128-chunk segment one-hot [edge, node-in-window]
            seg_ohs = []
            for j in range(cfg.ST // 128):
                c = t * (cfg.ST // 128) + j
                so = sohp.tile([128, 128], bf16, tag="soh")
                nc.vector.tensor_scalar(so[:], c_iota_t[:], c_wrc[:, c:c + 1],
                                        None, eq)
                seg_ohs.append(so)

            # hd via one-hot matmul, in 512-col halves
            for hhalf in range(cfg.ST // 512):
                cl0 = hhalf * 512
                bc = ps_bc.tile([128, 512], f32, tag="bc")
                nc.tensor.matmul(bc[:], c_ones1[:],
                                 wrr[:, cl0:cl0 + 512], start=True, stop=True)
                ohT = sb.tile([128, 512], bf16, tag="ohT")
                nc.vector.tensor_scalar(ohT[:], bc[:], c_iota_c[:], None, eq)
                hd = ps_hd.tile([64, 512], f32, tag="hd")
                # window-parts inside this half (chunks are window-pure)
                j0 = cl0 // 128
                parts = []
                for j in range(j0, j0 + 4):
                    c = t * (cfg.ST // 128) + j
                    w = int(plan.wchunk[c])
                    if parts and parts[-1][2] == w:
                        parts[-1][1] += 128
                    else:
                        parts.append([j * 128 - cl0, 128, w])
                for (o, wd, w) in parts:
                    nc.tensor.matmul(hd[:, o:o + wd],
                                     c_hwin[:, w * 64:(w + 1) * 64],
                                     ohT[:, o:o + wd], start=True, stop=True)
                # p = hs * hd  -> stack partitions 0:64
                nc.vector.tensor_tensor(
                    out=stack[0:64, cl0:cl0 + 512],
                    in0=hsb[:, cl0:cl0 + 512],
                    in1=hd[:, :], op=mul)

            z = ps_a.tile([128, cfg.ST], f32, tag="za")
            for hhalf in range(cfg.ST // 512):
                cl0 = hhalf * 512
                nc.tensor.matmul(z[:, cl0:cl0 + 512], c_wzp[:],
                                 stack[:, cl0:cl0 + 512], start=True, stop=True)
            rz = sb.tile([128, cfg.ST], bf16, tag="rz")
            nc.scalar.activation(rz[:], z[:], Relu, bias=c_be1[:, 0:1])

            m2 = ps_b.tile([128, cfg.ST], f32, tag="zb")
            for hhalf in range(cfg.ST // 512):
                cl0 = hhalf * 512
                nc.tensor.matmul(m2[:, cl0:cl0 + 512], c_we2[:],
                                 rz[:, cl0:cl0 + 512], start=True, stop=True)

            m2c = sb.tile([128, cfg.ST], bf16, tag="m2c")
            nc.scalar.activation(m2c[:], m2[:],
                                 mybir.ActivationFunctionType.Copy)
            q = sb.tile([128, cfg.ST], bf16, tag="q")
            nc.gpsimd.tensor_tensor(out=q[:, :], in0=m1sb[:, :],
                                    in1=m2c[:, :], op=mul)

            mnt = ps_mn.tile([128, cfg.ST // 128, 64], f32, tag="mnt")
            for j in range(cfg.ST // 128):
                nc.tensor.matmul(mnt[:, j, :], q[:, j * 128:(j + 1) * 128],
                                 c_wcomb[:], start=True, stop=True)
            msb = sb.tile([128, cfg.ST // 128, 64], bf16, tag="msb")
            nc.scalar.activation(msb[:], mnt[:], Tanh)

            for j in range(cfg.ST // 128):
                c = t * (cfg.ST // 128) + j
                w = int(plan.wchunk[c])
                first = bool(plan.first_chunk[c])
                last = bool(plan.last_chunk[c])
                slot = w % 8
                nc.tensor.matmul(aggp[:, slot, :], seg_ohs[j][:],
                                 msb[:, j, :], start=first, stop=last)
                if last:
                    nc.vector.tensor_copy(out=agg_sb[:, w * 64:(w + 1) * 64],
                                          in_=aggp[:, slot, :])

        agg_bf = pers.tile([128, NW * 64], bf16)
        nc.vector.tensor_copy(out=agg_bf[:], in_=agg_sb[:])
        nc.sync.dma_start(out=d_agg, in_=agg_bf[:])

    nc.compile()
    return nc


# --------------------------------------------------------------------------
# cached PJRT runner
# --------------------------------------------------------------------------

_BUNDLE: dict = {}


class _Runner:
    def __init__(self, nc, n_cores: int):
        import jax
        from jax.sharding import Mesh, PartitionSpec
        from jax.experimental.shard_map import shard_map
        from concourse import bass2jax

        bass2jax.install_neuronx_cc_hook()
        self.nc = nc
        self.n_cores = n_cores
        partition_name = (nc.partition_id_tensor.name
                          if nc.partition_id_tensor else None)
        assert nc.dbg_addr is None

        in_names, out_names, out_avals = [], [], []
        for alloc in nc.m.functions[0].allocations:
            if not isinstance(alloc, mybir.MemoryLocationSet):
                continue
            name = alloc.memorylocations[0].name
            if alloc.kind == "ExternalInput":
                if name != partition_name:
                    in_names.append(name)
            elif alloc.kind == "ExternalOutput":
                out_names.append(name)
                shape = tuple(alloc.tensor_shape)
                dtype = mybir.dt.np(alloc.dtype)
                out_avals.append(jax.core.ShapedArray(shape, dtype))
        n_params = len(in_names)
        all_names = list(in_names) + list(out_names)
        if partition_name is not None:
            all_names.append(partition_name)

        def _body(*args):
            operands = list(args)
            if partition_name is not None:
                operands.append(bass2jax.partition_id_tensor())
            outs = bass2jax._bass_exec_p.bind(
                *operands,
                out_avals=tuple(out_avals),
                in_names=tuple(all_names),
                out_names=tuple(out_names),
                lowering_input_output_aliases=(),
                sim_require_finite=True,
                sim_require_nnan=True,
                nc=nc,
            )
            return tuple(outs)

        devices = jax.devices()[:n_cores]
        mesh = Mesh(np.asarray(devices), ("core",))
        n_outs = len(out_names)
        in_specs = (PartitionSpec("core"),) * (n_params + n_outs)
        out_specs = (PartitionSpec("core"),) * n_outs
        self.fn = jax.jit(
            shard_map(_body, mesh=mesh, in_specs=in_specs,
                      out_specs=out_specs, check_rep=False),
            keep_unused=True,
        )
        # The trailing per-output operands only exist for the pre-zeroed
        # output convention; the kernel writes every output element, so ship
        # them to the devices once and reuse across calls (not donated).
        from jax.sharding import NamedSharding
        sh = NamedSharding(mesh, PartitionSpec("core"))
        self._zeros_dev = [
            jax.device_put(
                np.zeros((n_cores * a.shape[0], *a.shape[1:]), a.dtype), sh)
            for a in out_avals
        ]
        self.in_names = in_names
        self.out_names = out_names
        self.out_avals = out_avals

    def __call__(self, gmap: dict) -> dict:
        ins = [gmap[name] for name in self.in_names]
        outs = self.fn(*ins, *self._zeros_dev)
        return {name: np.asarray(a) for name, a in zip(self.out_names, outs)}


def _get_runner(cfg: Cfg, plan: Plan) -> _Runner:
    hsh = hashlib.sha1()
    hsh.update(plan.budg.tobytes())
    hsh.update(plan.pos0.tobytes())
    key = (cfg, plan.ET, hsh.hexdigest())
    r = _BUNDLE.get(key)
    if r is None:
        nc = _build(cfg, plan)
        r = _Runner(nc, cfg.NC)
        _BUNDLE[key] = r
    return r


# --------------------------------------------------------------------------
# entry points
# --------------------------------------------------------------------------

def _assemble(cfg: Cfg, agg_global: np.ndarray, ctx):
    h = ctx["h"]
    out = np.empty((cfg.N, cfg.DN), np.float32)
    for k in range(cfg.NC):
        agg = agg_global[k * 128:(k + 1) * 128].astype(np.float32)
        agg = agg.reshape(128, cfg.NW, 64).transpose(1, 0, 2).reshape(cfg.NW * 128, 64)
        out[k * cfg.NR:(k + 1) * cfg.NR] = agg[:cfg.NR] + h[k * cfg.NR:(k + 1) * cfg.NR]
    return out


def run_pipeline(cfg: Cfg, inputs: dict, backend: str = "hw"):
    src = np.asarray(inputs["src"]).astype(np.int64)
    dst = np.asarray(inputs["dst"]).astype(np.int64)
    plan = _make_plan(cfg, src, dst)
    gmap, ctx = _prep(cfg, inputs, plan)
    if backend == "sim":
        from concourse.bass_interp import CoreSim
        nc = _build(cfg, plan)
        aggs = []
        for k in range(cfg.NC):
            sim = CoreSim(nc, trace=False)
            for name, arr in gmap.items():
                d0 = arr.shape[0] // cfg.NC
                sim.tensor(name)[:] = arr[k * d0:(k + 1) * d0]
            sim.simulate()
            aggs.append(np.array(sim.tensor("agg")))
        return _assemble(cfg, np.concatenate(aggs, axis=0), ctx)
    runner = _get_runner(cfg, plan)
    res = runner(gmap)
    return _assemble(cfg, res["agg"], ctx)


def kernel(**inputs) -> np.ndarray:
    return run_pipeline(CFG_FULL, inputs, backend="hw")


if __name__ == "__main__":
    # smoke test at small scale on the simulator
    cfg = Cfg(N=2048, E=8192, NC=2, ST=1024)
    rng = np.random.default_rng(0)
    inputs = {
        "h": rng.standard_normal((cfg.N, 64)).astype(np.float32),
        "eh": rng.standard_normal((cfg.E, 64)).astype(np.float32),
        "W_node1": (rng.standard_normal((64, 128)) * 0.05).astype(np.float32),
        "b_node1": (rng.standard_normal(128) * 0.05).astype(np.float32),
        "W_node2": (rng.standard_normal((128, 128)) * 0.05).astype(np.float32),
        "W_edge1": (rng.standard_normal((64, 128)) * 0.05).astype(np.float32),
        "b_edge1": (rng.standard_normal(128) * 0.05).astype(np.float32),
        "W_edge2": (rng.standard_normal((128, 128)) * 0.05).astype(np.float32),
        "W_comb": (rng.standard_normal((128, 64)) * 0.05).astype(np.float32),
        "W_ue": (rng.standard_normal((64, 64)) * 0.05).astype(np.float32),
        "src": rng.integers(0, cfg.N, cfg.E).astype(np.int32),
        "dst": rng.integers(0, cfg.N, cfg.E).astype(np.int32),
    }
    h, eh = inputs["h"], inputs["eh"]
    hs, hd = h[inputs["src"]], h[inputs["dst"]]
    eh_new = 0.8 * eh + 0.2 * ((hs * hd) @ inputs["W_ue"])
    m1 = np.maximum(hs @ inputs["W_node1"] + inputs["b_node1"], 0) @ inputs["W_node2"]
    m2 = np.maximum(eh_new @ inputs["W_edge1"] + inputs["b_edge1"], 0) @ inputs["W_edge2"]
    m = np.tanh((m1 * m2) @ inputs["W_comb"])
    agg = np.zeros((cfg.N, 64), np.float32)
    np.add.at(agg, inputs["dst"], m)
    expected = agg + h

    out = run_pipeline(cfg, inputs, backend="sim")
    err = np.abs(out - expected)
    rel = np.abs(err).max() / np.abs(expected).max()
    print("max abs err:", err.max(), " rel(absmax):", rel)
    print("mean abs err:", err.mean())
    assert rel < 2e-2, "accuracy failure"
    print("SIM OK")


# revision 11
# speedup vs baseline: 5.9152x; 1.1416x over previous
"""Trainium2 Bass kernel for nn_DMGCNLayer (GNN message passing layer).

Strategy (graph/data parallel over 8 NeuronCores):
  - Edges are bucketed by dst node range (6250 nodes per core) so each core
    produces a disjoint slice of the output -> no cross-core reduction.
  - Within a core, edges are ordered by 128-node dst window with uniform
    (max-over-cores) per-bucket budgets so all 8 cores execute one identical
    SPMD program; shortfall is padded with self-neutralizing edges (their
    window-relative dst is 200, which matches no one-hot column).
  - The wall-clock of a run is dominated by the ~65 MB/s axon tunnel, so the
    two big per-edge streams (eh and host-gathered h[src]) are shipped as
    int8 with the dequant scales folded into the bf16 weights on the host;
    the device only widens int8 -> bf16. Metadata rides as uint8 and the
    output aggregate returns as bf16 (residual +h is added on host in fp32).
  - h[dst] and the segment-sum are realized as one-hot matmuls on the tensor
    engine (edges are sorted by dst window), accumulating in fp32 PSUM.
  - The message MLPs run in transposed-activation form in bf16 with folded
    weights: m2 = relu(eh@(0.8 W_e1) + (hs*hd)@(0.2 W_ue@W_e1) + b_e1)@W_e2.
  - The PJRT executable (jit of the shard_map'd bass custom call) is cached
    at module level so repeat runs pay only input upload + execute + fetch.
"""

import hashlib
from contextlib import ExitStack
from dataclasses import dataclass

import numpy as np
import ml_dtypes

import concourse.bass as bass
import concourse.bacc as bacc
import concourse.mybir as mybir
import concourse.tile as tile

BF16 = ml_dtypes.bfloat16
PADVAL = 200.0  # window-relative dst for pad edges; matches no iota column


@dataclass(frozen=True)
class Cfg:
    N: int = 50000
    E: int = 800000
    DN: int = 64
    H: int = 128
    NC: int = 8          # cores
    ST: int = 1024       # supertile (edges per pipeline step)

    @property
    def NR(self):  # nodes per core
        return self.N // self.NC

    @property
    def NW(self):  # 128-node windows per core
        return -(-self.NR // 128)


CFG_FULL = Cfg()


# --------------------------------------------------------------------------
# planning (uniform across cores)
# --------------------------------------------------------------------------

@dataclass
class Plan:
    budg: np.ndarray      # [NW] edge budget per window, 128-mult
    pos0: np.ndarray      # [NW] start position of each bucket
    ET: int               # total positions per core (multiple of ST)
    wchunk: np.ndarray    # [ET//128] window id of each 128-chunk
    first_chunk: np.ndarray  # [ET//128] bool: first chunk of its window block
    last_chunk: np.ndarray   # [ET//128] bool: last chunk of its window block


def _make_plan(cfg: Cfg, src: np.ndarray, dst: np.ndarray) -> Plan:
    NR, NW = cfg.NR, cfg.NW
    core = dst // NR
    win = (dst % NR) // 128

    counts = np.zeros((cfg.NC, NW), np.int64)
    np.add.at(counts, (core, win), 1)
    budg = counts.max(axis=0)
    budg = ((budg + 127) // 128) * 128
    budg[NW - 1] += (-budg.sum()) % cfg.ST

    pos0 = np.zeros(NW, np.int64)
    off = 0
    for w in range(NW):
        pos0[w] = off
        off += budg[w]
    ET = int(off)
    assert ET % cfg.ST == 0

    nch = ET // 128
    wchunk = np.zeros(nch, np.int64)
    first_chunk = np.zeros(nch, bool)
    last_chunk = np.zeros(nch, bool)
    for w in range(NW):
        c0 = int(pos0[w]) // 128
        c1 = c0 + int(budg[w]) // 128
        wchunk[c0:c1] = w
        first_chunk[c0] = True
        last_chunk[c1 - 1] = True
    return Plan(budg, pos0, ET, wchunk, first_chunk, last_chunk)


# --------------------------------------------------------------------------
# host-side input preparation
# --------------------------------------------------------------------------

def _prep(cfg: Cfg, inputs: dict, plan: Plan):
    """Build the global (concat-over-cores along axis 0) input arrays."""
    h = np.asarray(inputs["h"], np.float32)
    eh = np.asarray(inputs["eh"], np.float32)
    src = np.asarray(inputs["src"]).astype(np.int64)
    dst = np.asarray(inputs["dst"]).astype(np.int64)
    W_node1 = np.asarray(inputs["W_node1"], np.float32)
    b_node1 = np.asarray(inputs["b_node1"], np.float32)
    W_node2 = np.asarray(inputs["W_node2"], np.float32)
    W_edge1 = np.asarray(inputs["W_edge1"], np.float32)
    b_edge1 = np.asarray(inputs["b_edge1"], np.float32)
    W_edge2 = np.asarray(inputs["W_edge2"], np.float32)
    W_comb = np.asarray(inputs["W_comb"], np.float32)
    W_ue = np.asarray(inputs["W_ue"], np.float32)

    NR, NW, ET, NC = cfg.NR, cfg.NW, plan.ET, cfg.NC

    # int4 quantization: per-node scales for h (src stream), per-edge for eh.
    # Device reconstructs true values ((nib-8)*s), so only the int8 h-window
    # table used for the h[dst] one-hot gather needs a weight fold (s_h).
    s_h = float(np.abs(h).max()) / 127.0
    q8_h = np.clip(np.round(h / s_h), -127, 127).astype(np.int8)     # [N, 64]
    s_hn = (np.abs(h).max(1) / 7.5).astype(BF16).astype(np.float32)  # [N]
    s_hn = np.maximum(s_hn, 1e-6)
    q4_h = (np.clip(np.round(h / s_hn[:, None]), -8, 7) + 8).astype(np.uint8)
    # node table of nibble pairs, features on partitions: row q packs
    # features (2q | 2q+1); consumers see features in PI order (hi half
    # = even features, lo half = odd), so weights are PI-permuted below.
    PI = np.concatenate([np.arange(0, 64, 2), np.arange(1, 64, 2)])
    tab_bytes = np.ascontiguousarray(((q4_h[:, 0::2] << 4) | q4_h[:, 1::2]).T)
    tabh = tab_bytes.view("<u4")  # [32, N//4]: word j = nodes 4j..4j+3
    s_ee = (np.abs(eh).max(1) / 7.5).astype(BF16).astype(np.float32)  # [E]
    s_ee = np.maximum(s_ee, 1e-6)
    q4_eh = (np.clip(np.round(eh / s_ee[:, None]), -8, 7) + 8).astype(np.uint8)

    A = (0.8 * W_edge1).astype(BF16)                      # acts on true eh
    W_ue1 = (s_h * 0.2 * (W_ue @ W_edge1))[PI].astype(BF16)  # acts on hs * q8_hd
    wzp = np.concatenate([W_ue1, A], axis=0)              # [128, H]
    wn1 = W_node1[PI].astype(BF16)                        # acts on true hs

    iota_t = np.ascontiguousarray(
        np.broadcast_to(np.arange(128, dtype=np.float32), (128, 128))).astype(BF16)
    iota_c = np.arange(128, dtype=np.float32).reshape(128, 1)
    ones1 = np.ones((1, 128), BF16)

    core = dst // NR
    win = (dst % NR) // 128

    g_eh = np.zeros((NC * 64, ET // 2), np.uint8)
    g_gidx = np.zeros((NC * 16, ET // 16), np.int16)
    g_shift = np.zeros((NC * 1, ET), np.uint8)
    g_seh = np.zeros((NC * 1, ET), BF16)
    g_shs = np.zeros((NC * 1, ET), BF16)
    g_wrc = np.full((NC * 128, ET // 128), int(PADVAL), np.uint8)
    g_wrr = np.full((NC * 1, ET), int(PADVAL), np.uint8)
    g_hwin = np.zeros((NC * 128, NW * 64), np.int8)
    NSTEP = ET // cfg.ST

    def arrange_scales(s_row):
        # per supertile: first ST/2 cols = even-edge scales, rest = odd
        sr = s_row.reshape(NSTEP, cfg.ST)
        return np.concatenate([sr[:, 0::2], sr[:, 1::2]], axis=1).reshape(ET)

    for k in range(NC):
        perm = np.full(ET, -1, np.int64)
        ek = np.nonzero(core == k)[0]
        key = win[ek]
        order = np.argsort(key, kind="stable")
        ek = ek[order]
        key = key[order]
        starts = plan.pos0[key]
        changes = np.r_[True, key[1:] != key[:-1]]
        grp_start_idx = np.r_[0, np.nonzero(changes)[0][1:]]
        grp_id = np.cumsum(changes) - 1
        rank = np.arange(len(ek)) - grp_start_idx[grp_id]
        perm[starts + rank] = ek

        valid = perm >= 0
        pe = perm[valid]

        nib_e = np.full((64, ET), 8, np.uint8)
        nib_e[:, valid] = q4_eh[pe].T
        g_eh[k * 64:(k + 1) * 64] = (nib_e[:, 0::2] << 4) | nib_e[:, 1::2]
        se_row = np.ones(ET, np.float32)
        se_row[valid] = s_ee[pe]
        g_seh[k] = arrange_scales(se_row).astype(BF16)

        idx_vals = np.zeros(ET, np.int64)
        idx_vals[valid] = src[pe]
        g_gidx[k * 16:(k + 1) * 16] = (idx_vals >> 2).astype(np.int16).reshape(
            ET // 16, 16).T
        g_shift[k] = (8 * (idx_vals & 3)).astype(np.uint8)
        sh_row = np.ones(ET, np.float32)
        sh_row[valid] = s_hn[src[pe]]
        g_shs[k] = sh_row.astype(BF16)

        wrel = np.full(ET, int(PADVAL), np.uint8)
        wrel[valid] = (dst[pe] - k * NR - win[pe] * 128).astype(np.uint8)
        g_wrc[k * 128:(k + 1) * 128] = np.ascontiguousarray(
            wrel.reshape(ET // 128, 128).T)
        g_wrr[k] = wrel

        hk = q8_h[k * NR:(k + 1) * NR][:, PI]
        for w in range(NW):
            rows = hk[w * 128:(w + 1) * 128]
            g_hwin[k * 128:k * 128 + rows.shape[0], w * 64:w * 64 + 64] = rows

    def rep(x):  # replicate a shared tensor across cores along axis 0
        return np.ascontiguousarray(np.tile(x, (NC,) + (1,) * (x.ndim - 1)))

    gmap = {
        "eh_t": g_eh,
        "tabh": rep(tabh),
        "gidx": g_gidx,
        "shift": g_shift,
        "seh": g_seh,
        "shs": g_shs,
        "wrel_col": g_wrc,
        "wrel_row": g_wrr,
        "hwin": g_hwin,
        "wzp": rep(wzp),
        "we2": rep(W_edge2.astype(BF16)),
        "wcomb": rep(W_comb.astype(BF16)),
        "be1": rep(b_edge1.reshape(cfg.H, 1).astype(np.float32)),
        "iota_t": rep(iota_t),
        "iota_c": rep(iota_c),
        "ones1": rep(ones1),
        "wn1": rep(wn1),
        "wn2": rep(W_node2.astype(BF16)),
        "bn1": rep(b_node1.reshape(cfg.H, 1).astype(np.float32)),
    }
    ctx = {"h": h}
    return gmap, ctx


# --------------------------------------------------------------------------
# device program
# --------------------------------------------------------------------------

def _build(cfg: Cfg, plan: Plan) -> bacc.Bacc:
    ET, NW = plan.ET, cfg.NW
    f32 = mybir.dt.float32
    bf16 = mybir.dt.bfloat16
    i8 = mybir.dt.int8
    u8 = mybir.dt.uint8
    u16 = mybir.dt.uint16
    u32 = mybir.dt.uint32
    i16 = mybir.dt.int16

    nc = bacc.Bacc("TRN2", target_bir_lowering=False, debug=False,
                   enable_asserts=False)

    d_eh = nc.dram_tensor("eh_t", [64, ET // 2], u8, kind="ExternalInput").ap()
    d_tabh = nc.dram_tensor("tabh", [32, cfg.N // 4], u32, kind="ExternalInput").ap()
    d_gidx = nc.dram_tensor("gidx", [16, ET // 16], i16, kind="ExternalInput").ap()
    d_shift = nc.dram_tensor("shift", [1, ET], u8, kind="ExternalInput").ap()
    d_seh = nc.dram_tensor("seh", [1, ET], bf16, kind="ExternalInput").ap()
    d_shs = nc.dram_tensor("shs", [1, ET], bf16, kind="ExternalInput").ap()
    d_wn1 = nc.dram_tensor("wn1", [64, cfg.H], bf16, kind="ExternalInput").ap()
    d_wn2 = nc.dram_tensor("wn2", [cfg.H, cfg.H], bf16, kind="ExternalInput").ap()
    d_bn1 = nc.dram_tensor("bn1", [cfg.H, 1], f32, kind="ExternalInput").ap()
    d_wrc = nc.dram_tensor("wrel_col", [128, ET // 128], u8, kind="ExternalInput").ap()
    d_wrr = nc.dram_tensor("wrel_row", [1, ET], u8, kind="ExternalInput").ap()
    d_hwin = nc.dram_tensor("hwin", [128, NW * 64], i8, kind="ExternalInput").ap()
    d_wzp = nc.dram_tensor("wzp", [128, cfg.H], bf16, kind="ExternalInput").ap()
    d_we2 = nc.dram_tensor("we2", [cfg.H, cfg.H], bf16, kind="ExternalInput").ap()
    d_wcomb = nc.dram_tensor("wcomb", [cfg.H, 64], bf16, kind="ExternalInput").ap()
    d_be1 = nc.dram_tensor("be1", [cfg.H, 1], f32, kind="ExternalInput").ap()
    d_iota_t = nc.dram_tensor("iota_t", [128, 128], bf16, kind="ExternalInput").ap()
    d_iota_c = nc.dram_tensor("iota_c", [128, 1], f32, kind="ExternalInput").ap()
    d_ones1 = nc.dram_tensor("ones1", [1, 128], bf16, kind="ExternalInput").ap()
    d_agg = nc.dram_tensor("agg", [128, NW * 64], bf16, kind="ExternalOutput").ap()

    eq = mybir.AluOpType.is_equal
    mul = mybir.AluOpType.mult
    add = mybir.AluOpType.add
    shr = mybir.AluOpType.logical_shift_right
    band = mybir.AluOpType.bitwise_and
    Relu = mybir.ActivationFunctionType.Relu
    Tanh = mybir.ActivationFunctionType.Tanh

    NSTEP = ET // cfg.ST

    with tile.TileContext(nc) as tc, ExitStack() as ctx:
        con = ctx.enter_context(tc.tile_pool(name="const", bufs=1))
        sb = ctx.enter_context(tc.tile_pool(name="sb", bufs=2))
        sohp = ctx.enter_context(tc.tile_pool(name="soh", bufs=12))
        gpool = ctx.enter_context(tc.tile_pool(name="gbuf", bufs=2))
        pers = ctx.enter_context(tc.tile_pool(name="pers", bufs=1))
        ps_a = ctx.enter_context(tc.tile_pool(name="ps_a", bufs=1, space="PSUM"))
        ps_b = ctx.enter_context(tc.tile_pool(name="ps_b", bufs=1, space="PSUM"))
        ps_hd = ctx.enter_context(tc.tile_pool(name="ps_hd", bufs=1, space="PSUM"))
        ps_bc = ctx.enter_context(tc.tile_pool(name="ps_bc", bufs=1, space="PSUM"))
        ps_mn = ctx.enter_context(tc.tile_pool(name="ps_mn", bufs=1, space="PSUM"))
        ps_ag = ctx.enter_context(tc.tile_pool(name="ps_ag", bufs=1, space="PSUM"))

        def load_const(tag, dram_ap, shape, dtype):
            t_ = con.tile(shape, dtype, tag=tag)
            nc.sync.dma_start(out=t_[:], in_=dram_ap)
            return t_

        c_wzp = load_const("wzp", d_wzp, [128, cfg.H], bf16)
        c_we2 = load_const("we2", d_we2, [cfg.H, cfg.H], bf16)
        c_wcomb = load_const("wcomb", d_wcomb, [cfg.H, 64], bf16)
        c_be1 = load_const("be1", d_be1, [cfg.H, 1], f32)
        c_iota_t = load_const("iota_t", d_iota_t, [128, 128], bf16)
        c_iota_c = load_const("iota_c", d_iota_c, [128, 1], f32)
        c_ones1 = load_const("ones1", d_ones1, [1, 128], bf16)
        c_hwin8 = load_const("hwin8", d_hwin, [128, NW * 64], i8)
        c_wrc8 = load_const("wrc8", d_wrc, [128, ET // 128], u8)
        c_wn1 = load_const("wn1", d_wn1, [64, cfg.H], bf16)
        c_wn2 = load_const("wn2", d_wn2, [cfg.H, cfg.H], bf16)
        c_bn1 = load_const("bn1", d_bn1, [cfg.H, 1], f32)

        c_wrc = pers.tile([128, ET // 128], f32)
        nc.vector.tensor_copy(out=c_wrc[:], in_=c_wrc8[:])
        c_hwin = pers.tile([128, NW * 64], bf16)
        nc.vector.tensor_copy(out=c_hwin[:], in_=c_hwin8[:])
        # node table for the on-device h[src] gather: u32 words of 4
        # nibble-pair bytes, features (pairs) on partitions 0:32
        c_tab = pers.tile([32, cfg.N // 4], u32)
        nc.sync.dma_start(out=c_tab[:], in_=d_tabh)
        c_gidx = pers.tile([32, ET // 16], i16)
        nc.sync.dma_start(out=c_gidx[0:16, :], in_=d_gidx)
        nc.sync.dma_start(out=c_gidx[16:32, :], in_=d_gidx)

        agg_sb = pers.tile([128, NW * 64], f32)
        aggp = ps_ag.tile([128, 8, 64], f32)  # rotating window accumulators

        HT = cfg.ST // 2

        def bcast_dma(out_tile, dram_row_ap, nparts):
            ap = dram_row_ap
            ap.ap = [[0, nparts]] + ap.ap[1:]
            nc.sync.dma_start(out=out_tile, in_=ap)

        for t in range(NSTEP):
            g32 = gpool.tile([32, cfg.ST], u32, tag="g32")
            nc.gpsimd.ap_gather(
                g32[:], c_tab[:],
                c_gidx[:, t * (cfg.ST // 16):(t + 1) * (cfg.ST // 16)],
                channels=32, num_elems=cfg.N // 4, d=1, num_idxs=cfg.ST)
            shb8 = gpool.tile([32, cfg.ST], u8, tag="shb8")
            bcast_dma(shb8[:], d_shift[0:1, t * cfg.ST:(t + 1) * cfg.ST], 32)
            shi = gpool.tile([32, cfg.ST], u32, tag="shi")
            nc.vector.tensor_copy(out=shi[:], in_=shb8[:])
            word = gpool.tile([32, cfg.ST], u32, tag="word")
            nc.vector.tensor_tensor(out=word[:], in0=g32[:], in1=shi[:], op=shr)
            hs_hi = gpool.tile([32, cfg.ST], u32, tag="hshi")
            nc.vector.tensor_scalar(hs_hi[:], word[:], 255, 4,
                                    band, shr)
            hs_lo = gpool.tile([32, cfg.ST], u32, tag="hslo")
            nc.vector.tensor_scalar(hs_lo[:], word[:], 15, None, band)
            bc_hs = gpool.tile([32, cfg.ST], bf16, tag="bchs")
            bcast_dma(bc_hs[:], d_shs[0:1, t * cfg.ST:(t + 1) * cfg.ST], 32)
            hsb = gpool.tile([64, cfg.ST], bf16, tag="hsb")
            nc.vector.scalar_tensor_tensor(out=hsb[0:32, :], in0=hs_hi[:],
                                           scalar=-8.0, in1=bc_hs[:],
                                           op0=add, op1=mul)
            nc.vector.scalar_tensor_tensor(out=hsb[32:64, :], in0=hs_lo[:],
                                           scalar=-8.0, in1=bc_hs[:],
                                           op0=add, op1=mul)

            # per-edge MLP1: m1 = relu(hs@Wn1 + bn1)@Wn2, in transposed form
            z1 = ps_a.tile([128, cfg.ST], f32, tag="za")
            for hhalf in range(cfg.ST // 512):
                cl0 = hhalf * 512
                nc.tensor.matmul(z1[:, cl0:cl0 + 512], c_wn1[:],
                                 hsb[:, cl0:cl0 + 512],
                                 start=True, stop=True)
            r1 = sb.tile([128, cfg.ST], bf16, tag="r1")
            nc.vector.tensor_scalar(r1[:], z1[:], c_bn1[:, 0:1], 0.0,
                                    mybir.AluOpType.add, mybir.AluOpType.max)
            m1p = ps_b.tile([128, cfg.ST], f32, tag="zb")
            for hhalf in range(cfg.ST // 512):
                cl0 = hhalf * 512
                nc.tensor.matmul(m1p[:, cl0:cl0 + 512], c_wn2[:],
                                 r1[:, cl0:cl0 + 512], start=True, stop=True)
            m1sb = sb.tile([128, cfg.ST], bf16, tag="m1sb")
            nc.vector.tensor_copy(out=m1sb[:], in_=m1p[:])

            stack = sb.tile([128, cfg.ST], bf16, tag="stack")
            eh4 = gpool.tile([64, HT], u8, tag="eh4")
            nc.sync.dma_start(out=eh4[:], in_=d_eh[:, t * HT:(t + 1) * HT])
            bc_ee = gpool.tile([64, HT], bf16, tag="bcee")
            bcast_dma(bc_ee[:], d_seh[0:1, t * cfg.ST:t * cfg.ST + HT], 64)
            bc_eo = gpool.tile([64, HT], bf16, tag="bceo")
            bcast_dma(bc_eo[:], d_seh[0:1, t * cfg.ST + HT:(t + 1) * cfg.ST], 64)
            eh_hi = gpool.tile([64, HT], u8, tag="ehhi")
            nc.vector.tensor_scalar(eh_hi[:], eh4[:], 4, None, shr)
            eh_lo = gpool.tile([64, HT], u8, tag="ehlo")
            nc.vector.tensor_scalar(eh_lo[:], eh4[:], 15, None, band)
            nc.vector.scalar_tensor_tensor(out=stack[64:128, 0:cfg.ST:2],
                                           in0=eh_hi[:], scalar=-8.0,
                                           in1=bc_ee[:], op0=add, op1=mul)
            nc.vector.scalar_tensor_tensor(out=stack[64:128, 1:cfg.ST:2],
                                           in0=eh_lo[:], scalar=-8.0,
                                           in1=bc_eo[:], op0=add, op1=mul)
            wrr8 = sb.tile([1, cfg.ST], u8, tag="wrr8")
            nc.sync.dma_start(out=wrr8[:], in_=d_wrr[:, t * cfg.ST:(t + 1) * cfg.ST])
            wrr = sb.tile([1, cfg.ST], bf16, tag="wrr")
            nc.vector.tensor_copy(out=wrr[:], in_=wrr8[:])

            # per-128-chunk segment one-hot [edge, node-in-window]
            seg_ohs = []
            for j in range(cfg.ST // 128):
                c = t * (cfg.ST // 128) + j
                so = sohp.tile([128, 128], bf16, tag="soh")
                nc.vector.tensor_scalar(so[:], c_iota_t[:], c_wrc[:, c:c + 1],
                                        None, eq)
                seg_ohs.append(so)

            # hd via one-hot matmul, in 512-col halves
            for hhalf in range(cfg.ST // 512):
                cl0 = hhalf * 512
                bc = ps_bc.tile([128, 512], f32, tag="bc")
                nc.tensor.matmul(bc[:], c_ones1[:],
                                 wrr[:, cl0:cl0 + 512], start=True, stop=True)
                ohT = sb.tile([128, 512], bf16, tag="ohT")
                nc.vector.tensor_scalar(ohT[:], bc[:], c_iota_c[:], None, eq)
                hd = ps_hd.tile([64, 512], f32, tag="hd")
                # window-parts inside this half (chunks are window-pure)
                j0 = cl0 // 128
                parts = []
                for j in range(j0, j0 + 4):
                    c = t * (cfg.ST // 128) + j
                    w = int(plan.wchunk[c])
                    if parts and parts[-1][2] == w:
                        parts[-1][1] += 128
                    else:
                        parts.append([j * 128 - cl0, 128, w])
                for (o, wd, w) in parts:
                    nc.tensor.matmul(hd[:, o:o + wd],
                                     c_hwin[:, w * 64:(w + 1) * 64],
                                     ohT[:, o:o + wd], start=True, stop=True)
                # p = hs * hd  -> stack partitions 0:64
                nc.vector.tensor_tensor(
                    out=stack[0:64, cl0:cl0 + 512],
                    in0=hsb[:, cl0:cl0 + 512],
                    in1=hd[:, :], op=mul)

            z = ps_a.tile([128, cfg.ST], f32, tag="za")
            for hhalf in range(cfg.ST // 512):
                cl0 = hhalf * 512
                nc.tensor.matmul(z[:, cl0:cl0 + 512], c_wzp[:],
                                 stack[:, cl0:cl0 + 512], start=True, stop=True)
            rz = sb.tile([128, cfg.ST], bf16, tag="rz")
            nc.scalar.activation(rz[:], z[:], Relu, bias=c_be1[:, 0:1])

            m2 = ps_b.tile([128, cfg.ST], f32, tag="zb")
            for hhalf in range(cfg.ST // 512):
                cl0 = hhalf * 512
                nc.tensor.matmul(m2[:, cl0:cl0 + 512], c_we2[:],
                                 rz[:, cl0:cl0 + 512], start=True, stop=True)

            m2c = sb.tile([128, cfg.ST], bf16, tag="m2c")
            nc.scalar.activation(m2c[:], m2[:],
                                 mybir.ActivationFunctionType.Copy)
            q = sb.tile([128, cfg.ST], bf16, tag="q")
            nc.vector.tensor_tensor(out=q[:, :], in0=m1sb[:, :],
                                    in1=m2c[:, :], op=mul)

            mnt = ps_mn.tile([128, cfg.ST // 128, 64], f32, tag="mnt")
            for j in range(cfg.ST // 128):
                nc.tensor.matmul(mnt[:, j, :], q[:, j * 128:(j + 1) * 128],
                                 c_wcomb[:], start=True, stop=True)
            msb = sb.tile([128, cfg.ST // 128, 64], bf16, tag="msb")
            nc.scalar.activation(msb[:], mnt[:], Tanh)

            for j in range(cfg.ST // 128):
                c = t * (cfg.ST // 128) + j
                w = int(plan.wchunk[c])
                first = bool(plan.first_chunk[c])
                last = bool(plan.last_chunk[c])
                slot = w % 8
                nc.tensor.matmul(aggp[:, slot, :], seg_ohs[j][:],
                                 msb[:, j, :], start=first, stop=last)
                if last:
                    nc.vector.tensor_copy(out=agg_sb[:, w * 64:(w + 1) * 64],
                                          in_=aggp[:, slot, :])

        agg_bf = pers.tile([128, NW * 64], bf16)
        nc.vector.tensor_copy(out=agg_bf[:], in_=agg_sb[:])
        nc.sync.dma_start(out=d_agg, in_=agg_bf[:])

    nc.compile()
    return nc


# --------------------------------------------------------------------------
# cached PJRT runner
# --------------------------------------------------------------------------

_BUNDLE: dict = {}


class _Runner:
    def __init__(self, nc, n_cores: int):
        import jax
        from jax.sharding import Mesh, PartitionSpec
        from jax.experimental.shard_map import shard_map
        from concourse import bass2jax

        bass2jax.install_neuronx_cc_hook()
        self.nc = nc
        self.n_cores = n_cores
        partition_name = (nc.partition_id_tensor.name
                          if nc.partition_id_tensor else None)
        assert nc.dbg_addr is None

        in_names, out_names, out_avals = [], [], []
        for alloc in nc.m.functions[0].allocations:
            if not isinstance(alloc, mybir.MemoryLocationSet):
                continue
            name = alloc.memorylocations[0].name
            if alloc.kind == "ExternalInput":
                if name != partition_name:
                    in_names.append(name)
            elif alloc.kind == "ExternalOutput":
                out_names.append(name)
                shape = tuple(alloc.tensor_shape)
                dtype = mybir.dt.np(alloc.dtype)
                out_avals.append(jax.core.ShapedArray(shape, dtype))
        n_params = len(in_names)
        all_names = list(in_names) + list(out_names)
        if partition_name is not None:
            all_names.append(partition_name)

        def _body(*args):
            operands = list(args)
            if partition_name is not None:
                operands.append(bass2jax.partition_id_tensor())
            outs = bass2jax._bass_exec_p.bind(
                *operands,
                out_avals=tuple(out_avals),
                in_names=tuple(all_names),
                out_names=tuple(out_names),
                lowering_input_output_aliases=(),
                sim_require_finite=True,
                sim_require_nnan=True,
                nc=nc,
            )
            return tuple(outs)

        devices = jax.devices()[:n_cores]
        mesh = Mesh(np.asarray(devices), ("core",))
        n_outs = len(out_names)
        in_specs = (PartitionSpec("core"),) * (n_params + n_outs)
        out_specs = (PartitionSpec("core"),) * n_outs
        self.fn = jax.jit(
            shard_map(_body, mesh=mesh, in_specs=in_specs,
                      out_specs=out_specs, check_rep=False),
            keep_unused=True,
        )
        # The trailing per-output operands only exist for the pre-zeroed
        # output convention; the kernel writes every output element, so ship
        # them to the devices once and reuse across calls (not donated).
        from jax.sharding import NamedSharding
        sh = NamedSharding(mesh, PartitionSpec("core"))
        self._zeros_dev = [
            jax.device_put(
                np.zeros((n_cores * a.shape[0], *a.shape[1:]), a.dtype), sh)
            for a in out_avals
        ]
        self.in_names = in_names
        self.out_names = out_names
        self.out_avals = out_avals

    def __call__(self, gmap: dict) -> dict:
        ins = [gmap[name] for name in self.in_names]
        outs = self.fn(*ins, *self._zeros_dev)
        return {name: np.asarray(a) for name, a in zip(self.out_names, outs)}


def _get_runner(cfg: Cfg, plan: Plan) -> _Runner:
    hsh = hashlib.sha1()
    hsh.update(plan.budg.tobytes())
    hsh.update(plan.pos0.tobytes())
    key = (cfg, plan.ET, hsh.hexdigest())
    r = _BUNDLE.get(key)
    if r is None:
        nc = _build(cfg, plan)
        r = _Runner(nc, cfg.NC)
        _BUNDLE[key] = r
    return r


# --------------------------------------------------------------------------
# entry points
# --------------------------------------------------------------------------

def _assemble(cfg: Cfg, agg_global: np.ndarray, ctx):
    h = ctx["h"]
    out = np.empty((cfg.N, cfg.DN), np.float32)
    for k in range(cfg.NC):
        agg = agg_global[k * 128:(k + 1) * 128].astype(np.float32)
        agg = agg.reshape(128, cfg.NW, 64).transpose(1, 0, 2).reshape(cfg.NW * 128, 64)
        out[k * cfg.NR:(k + 1) * cfg.NR] = agg[:cfg.NR] + h[k * cfg.NR:(k + 1) * cfg.NR]
    return out


def run_pipeline(cfg: Cfg, inputs: dict, backend: str = "hw"):
    src = np.asarray(inputs["src"]).astype(np.int64)
    dst = np.asarray(inputs["dst"]).astype(np.int64)
    plan = _make_plan(cfg, src, dst)
    gmap, ctx = _prep(cfg, inputs, plan)
    if backend == "sim":
        from concourse.bass_interp import CoreSim
        nc = _build(cfg, plan)
        aggs = []
        for k in range(cfg.NC):
            sim = CoreSim(nc, trace=False)
            for name, arr in gmap.items():
                d0 = arr.shape[0] // cfg.NC
                sim.tensor(name)[:] = arr[k * d0:(k + 1) * d0]
            sim.simulate()
            aggs.append(np.array(sim.tensor("agg")))
        return _assemble(cfg, np.concatenate(aggs, axis=0), ctx)
    runner = _get_runner(cfg, plan)
    res = runner(gmap)
    return _assemble(cfg, res["agg"], ctx)


def kernel(**inputs) -> np.ndarray:
    return run_pipeline(CFG_FULL, inputs, backend="hw")


if __name__ == "__main__":
    # smoke test at small scale on the simulator
    cfg = Cfg(N=2048, E=8192, NC=2, ST=1024)
    rng = np.random.default_rng(0)
    inputs = {
        "h": rng.standard_normal((cfg.N, 64)).astype(np.float32),
        "eh": rng.standard_normal((cfg.E, 64)).astype(np.float32),
        "W_node1": (rng.standard_normal((64, 128)) * 0.05).astype(np.float32),
        "b_node1": (rng.standard_normal(128) * 0.05).astype(np.float32),
        "W_node2": (rng.standard_normal((128, 128)) * 0.05).astype(np.float32),
        "W_edge1": (rng.standard_normal((64, 128)) * 0.05).astype(np.float32),
        "b_edge1": (rng.standard_normal(128) * 0.05).astype(np.float32),
        "W_edge2": (rng.standard_normal((128, 128)) * 0.05).astype(np.float32),
        "W_comb": (rng.standard_normal((128, 64)) * 0.05).astype(np.float32),
        "W_ue": (rng.standard_normal((64, 64)) * 0.05).astype(np.float32),
        "src": rng.integers(0, cfg.N, cfg.E).astype(np.int32),
        "dst": rng.integers(0, cfg.N, cfg.E).astype(np.int32),
    }
    h, eh = inputs["h"], inputs["eh"]
    hs, hd = h[inputs["src"]], h[inputs["dst"]]
    eh_new = 0.8 * eh + 0.2 * ((hs * hd) @ inputs["W_ue"])
    m1 = np.maximum(hs @ inputs["W_node1"] + inputs["b_node1"], 0) @ inputs["W_node2"]
    m2 = np.maximum(eh_new @ inputs["W_edge1"] + inputs["b_edge1"], 0) @ inputs["W_edge2"]
    m = np.tanh((m1 * m2) @ inputs["W_comb"])
    agg = np.zeros((cfg.N, 64), np.float32)
    np.add.at(agg, inputs["dst"], m)
    expected = agg + h

    out = run_pipeline(cfg, inputs, backend="sim")
    err = np.abs(out - expected)
    rel = np.abs(err).max() / np.abs(expected).max()
    print("max abs err:", err.max(), " rel(absmax):", rel)
    print("mean abs err:", err.mean())
    assert rel < 2e-2, "accuracy failure"
    print("SIM OK")
